# revision 8
# baseline (speedup 1.0000x reference)
"""Trainium2 Bass kernel for the DiscreteAgent GNN (NNConv + LN + MLP head).

Strategy (8 NeuronCores, SPMD, no collectives):
  * Edges bucketed by destination 128-node block; each core owns a disjoint
    6250-node range so outputs never overlap -> no all-reduce.
  * Within a core, block slots are ordered by descending edge count and the
    per-slot tile capacity is the max across cores, so the SPMD program is
    shared while padding stays small.
  * Per 128-edge tile: PE computes w_pre = [attr|1]^T @ [We;be] (512 cols),
    ACT/GPSIMD/DVE apply relu (split by tile for balance), DVE multiplies by
    the broadcast source features, and the i-contraction is FUSED into the
    scatter: 16 chained 32-col matmuls (one per input channel) accumulate
    onehot^T @ prod[:, :, i] into the block's [128, 32] PSUM slice.  The
    root-weight matmul opens each block's accumulation chain, so
    h = agg + x@Wroot + bconv materializes directly in PSUM.
  * One-hot scatter matrices are precomputed on the host and DMA'd (no
    per-tile is_equal on DVE).
  * Node phase per 4-block super-block: batched LN stats, per-block
    relu((h-mu)*rstd) fused into one ACT op (scale/bias per-partition),
    batched PE transpose, then the 3-layer MLP head in 2-block chunks with
    dual-op PSUM drains.
"""

import sys

import numpy as np

try:
    import concourse  # noqa: F401
except ImportError:  # pragma: no cover
    for _p in ("/opt/trn_rl_repo", "/opt/pypackages"):
        if _p not in sys.path:
            sys.path.insert(0, _p)

# ---- problem constants (hardcoded per contract) ----
N = 50000
E = 200000
IN_C = 16
HID_C = 32
EDGE_D = 8
OUT_C = 32
MLP_H = 128
N_ACT = 32

M = 8                 # cores
P = 128               # partitions
NPC = N // M          # 6250 nodes per core
NB = (NPC + P - 1) // P   # 49 block slots per core
NPC_PAD = NB * P      # 6272
G = 8                 # edge tiles per DMA group
SBW = 4               # blocks per node-phase super-block (stats/transpose)
CW = 2                # blocks per MLP chunk
EDGE_LAG = 4          # scatter trails the relu/mul front by this many tiles

# relu engine per tile, cycled: 'A'=ACT, 'V'=DVE (GPSIMD cannot read PSUM)
RELU_PATTERN = "AAVAAVAAVA"
# mul engine per tile: 'V'=DVE, 'G'=GPSIMD
MUL_PATTERN = "GV"

_PROGRAM_CACHE: dict = {}


def _build_program(kts: tuple, affine_ln: bool):
    """Build + compile the SPMD Bass program.

    kts: per-block-slot edge-tile counts (len NB, each >= 1).
    affine_ln: if True, gamma/beta are non-trivial and applied explicitly.
    """
    import concourse.tile as tile
    from concourse import bacc, mybir
    from concourse.masks import make_identity

    f32 = mybir.dt.float32
    fp16 = mybir.dt.float16
    Act = mybir.ActivationFunctionType
    Alu = mybir.AluOpType

    ET = int(sum(kts))            # edge tiles per core
    EPC = ET * P                  # padded edge slots per core
    offs = np.concatenate([[0], np.cumsum(kts)]).astype(int)  # tile offsets

    # flat tile -> (slot, kt) map
    tile_slot = np.empty(ET, int)
    tile_kt = np.empty(ET, int)
    for k in range(NB):
        tile_slot[offs[k]:offs[k + 1]] = k
        tile_kt[offs[k]:offs[k + 1]] = np.arange(kts[k])

    nc = bacc.Bacc("TRN2", target_bir_lowering=False, debug=False, num_devices=M)

    # --- DRAM I/O (per core) ---
    attrT = nc.dram_tensor("attrT", [EDGE_D + 1, EPC], fp16, kind="ExternalInput")
    xjg = nc.dram_tensor("xjg", [EPC, IN_C], fp16, kind="ExternalInput")
    ohg = nc.dram_tensor("ohg", [EPC, P], fp16, kind="ExternalInput")
    xsT = nc.dram_tensor("xsT", [IN_C + 1, NPC_PAD], fp16, kind="ExternalInput")
    weA = nc.dram_tensor("weA", [EDGE_D + 1, IN_C * HID_C], fp16, kind="ExternalInput")
    wrootA = nc.dram_tensor("wrootA", [IN_C + 1, HID_C], fp16, kind="ExternalInput")
    wlin = nc.dram_tensor("wlin", [HID_C, OUT_C], fp16, kind="ExternalInput")
    wq1 = nc.dram_tensor("wq1", [OUT_C, MLP_H], fp16, kind="ExternalInput")
    wq2 = nc.dram_tensor("wq2", [MLP_H, N_ACT], fp16, kind="ExternalInput")
    bq1c = nc.dram_tensor("bq1c", [MLP_H, 1], f32, kind="ExternalInput")
    bq2c = nc.dram_tensor("bq2c", [N_ACT, 1], f32, kind="ExternalInput")
    gamma4 = nc.dram_tensor("gamma4", [P, SBW * HID_C], fp16, kind="ExternalInput")
    beta4 = nc.dram_tensor("beta4", [P, SBW * HID_C], fp16, kind="ExternalInput")
    qT = nc.dram_tensor("qT", [N_ACT, NPC_PAD], f32, kind="ExternalOutput")

    NSB = (NB + SBW - 1) // SBW   # super-blocks

    with tile.TileContext(nc) as tc:
        with (
            tc.tile_pool(name="const", bufs=1) as cpool,
            tc.tile_pool(name="edge_in", bufs=3) as epool,
            tc.tile_pool(name="wrelu", bufs=6) as wpool,
            tc.tile_pool(name="node", bufs=2) as npool,
            tc.tile_pool(name="stats", bufs=2) as spool,
            tc.tile_pool(name="wpre_ps", bufs=2, space="PSUM") as wpre_ps,
            tc.tile_pool(name="agg_ps", bufs=2, space="PSUM") as agg_ps,
            tc.tile_pool(name="tr_ps", bufs=1, space="PSUM") as tr_ps,
            tc.tile_pool(name="mlp_ps", bufs=1, space="PSUM") as mlp_ps,
        ):
            group_state = {}
            NGROUPS = (ET + G - 1) // G

            def emit_group_load(g):
                gs = min(G, ET - g * G)
                esl = slice(g * G * P, (g * G + gs) * P)
                attr_g = epool.tile([EDGE_D + 1, G * P], fp16, tag="attr")
                nc.sync.dma_start(attr_g[:, :gs * P], attrT.ap()[:, esl])
                xj_g = epool.tile([P, G, IN_C], fp16, tag="xj")
                nc.sync.dma_start(
                    xj_g[:, :gs, :],
                    xjg.ap()[esl, :].rearrange("(tt p) i -> p tt i", p=P))
                oh_g = epool.tile([P, G, P], fp16, tag="oh")
                nc.sync.dma_start(
                    oh_g[:, :gs, :],
                    ohg.ap()[esl, :].rearrange("(tt p) n -> p tt n", p=P))
                group_state[g] = (attr_g, xj_g, oh_g)

            for g in range(2):
                emit_group_load(g)

            # ---- persistent constants in SBUF ----
            we_sb = cpool.tile([EDGE_D + 1, IN_C * HID_C], fp16, tag="we")
            nc.sync.dma_start(we_sb[:], weA.ap()[:])
            xsT_sb = cpool.tile([IN_C + 1, NPC_PAD], fp16, tag="xsT")
            nc.sync.dma_start(xsT_sb[:], xsT.ap()[:])
            wroot_sb = cpool.tile([IN_C + 1, HID_C], fp16, tag="wroot")
            nc.sync.dma_start(wroot_sb[:], wrootA.ap()[:])
            wlin_sb = cpool.tile([HID_C, OUT_C], fp16, tag="wlin")
            nc.sync.dma_start(wlin_sb[:], wlin.ap()[:])
            wq1_sb = cpool.tile([OUT_C, MLP_H], fp16, tag="wq1")
            nc.sync.dma_start(wq1_sb[:], wq1.ap()[:])
            wq2_sb = cpool.tile([MLP_H, N_ACT], fp16, tag="wq2")
            nc.sync.dma_start(wq2_sb[:], wq2.ap()[:])
            bq1_sb = cpool.tile([MLP_H, 1], f32, tag="bq1")
            nc.sync.dma_start(bq1_sb[:], bq1c.ap()[:])
            bq2_sb = cpool.tile([N_ACT, 1], f32, tag="bq2")
            nc.sync.dma_start(bq2_sb[:], bq2c.ap()[:])
            if affine_ln:
                gam_sb = cpool.tile([P, SBW * HID_C], fp16, tag="gam")
                nc.sync.dma_start(gam_sb[:], gamma4.ap()[:])
                bet_sb = cpool.tile([P, SBW * HID_C], fp16, tag="bet")
                nc.sync.dma_start(bet_sb[:], beta4.ap()[:])

            ident = cpool.tile([P, P], fp16, tag="ident")
            make_identity(nc, ident[:])
            eps_c = cpool.tile([P, 1], f32, tag="eps")
            nc.gpsimd.memset(eps_c[:], 1e-5)

            # ---- pipeline state ----
            edge_state = {}       # t -> prod tile
            agg_by_sb = {}        # sb -> agg psum tile [P, SBW*HID_C]
            sb_state = {}         # sb -> dict (stage A outputs)

            def emit_front(t):
                g, tt = divmod(t, G)
                if tt == 0 and g not in group_state:
                    emit_group_load(g)
                if tt == 0 and g + 1 < NGROUPS and g + 1 not in group_state:
                    emit_group_load(g + 1)
                attr_g, xj_g, oh_g = group_state[g]

                wpre = wpre_ps.tile([P, IN_C * HID_C], f32, tag="wpre")
                nc.tensor.matmul(wpre[:], lhsT=attr_g[:, tt * P:(tt + 1) * P],
                                 rhs=we_sb[:], start=True, stop=True)
                wrelu = wpool.tile([P, IN_C * HID_C], fp16, tag="wrelu")
                eng = RELU_PATTERN[t % len(RELU_PATTERN)]
                if eng == "A":
                    nc.scalar.activation(wrelu[:], wpre[:], Act.Relu)
                else:
                    nc.vector.tensor_scalar(wrelu[:], wpre[:], 0.0, None,
                                            op0=Alu.max)
                prod = wpool.tile([P, IN_C * HID_C], fp16, tag="prod")
                xj_b = xj_g[:, t % G, :].unsqueeze(1).to_broadcast(
                    [P, HID_C, IN_C])
                prod_3d = prod[:].rearrange("p (h i) -> p h i", h=HID_C)
                wrelu_3d = wrelu[:].rearrange("p (h i) -> p h i", h=HID_C)
                if MUL_PATTERN[t % len(MUL_PATTERN)] == "G":
                    nc.gpsimd.tensor_tensor(prod_3d, wrelu_3d, xj_b,
                                            op=Alu.mult)
                else:
                    nc.vector.tensor_tensor(prod_3d, wrelu_3d, xj_b,
                                            op=Alu.mult)
                edge_state[t] = (prod, g, tt)

            def emit_scatter(t):
                k = int(tile_slot[t])
                kt = int(tile_kt[t])
                sb, kb = divmod(k, SBW)
                prod, g, tt = edge_state.pop(t)
                oh_g = group_state[g][2]
                if kb == 0 and kt == 0:
                    agg_by_sb[sb] = agg_ps.tile([P, SBW * HID_C], f32,
                                                tag="agg", name="agg")
                agg = agg_by_sb[sb]
                out_sl = agg[:, kb * HID_C:(kb + 1) * HID_C]
                if kt == 0:
                    # open the accumulation chain with the root matmul
                    nsl = slice(k * P, (k + 1) * P)
                    nc.tensor.matmul(out_sl[:], lhsT=xsT_sb[:, nsl],
                                     rhs=wroot_sb[:], start=True, stop=False)
                prod_3d = prod[:].rearrange("p (h i) -> p h i", h=HID_C)
                last = (kt == kts[k] - 1)
                for i in range(IN_C):
                    nc.tensor.matmul(out_sl[:], lhsT=oh_g[:, tt, :],
                                     rhs=prod_3d[:, :, i],
                                     start=False, stop=(last and i == IN_C - 1))
                # release of group g handled by pool rotation

            def emit_node_a(sb):
                """LN stats + fused relu((h-mu)*rstd) + (optional gamma/beta)."""
                nblk = min(SBW, NB - sb * SBW)
                agg = agg_by_sb.pop(sb)
                h_all = agg[:, :nblk * HID_C]
                h3 = h_all[:].rearrange("p (b h) -> p b h", h=HID_C)
                st = {}
                musum = spool.tile([P, SBW], f32, tag="musum")
                nc.vector.tensor_reduce(musum[:, :nblk], h3,
                                        axis=mybir.AxisListType.X, op=Alu.add)
                hsq = wpool.tile([P, SBW * HID_C], fp16, tag="hsq")
                nc.scalar.activation(hsq[:, :nblk * HID_C], h_all[:], Act.Square)
                m2 = spool.tile([P, SBW], f32, tag="m2")
                nc.vector.tensor_reduce(
                    m2[:, :nblk],
                    hsq[:, :nblk * HID_C].rearrange("p (b h) -> p b h", h=HID_C),
                    axis=mybir.AxisListType.X, op=Alu.add)
                mu = spool.tile([P, SBW], f32, tag="mu")
                nc.vector.tensor_scalar(mu[:, :nblk], musum[:, :nblk],
                                        1.0 / HID_C, None, op0=Alu.mult)
                musq = spool.tile([P, SBW], f32, tag="musq")
                nc.vector.tensor_tensor(musq[:, :nblk], mu[:, :nblk],
                                        mu[:, :nblk], op=Alu.mult)
                m2n = spool.tile([P, SBW], f32, tag="m2n")
                nc.vector.tensor_scalar(m2n[:, :nblk], m2[:, :nblk],
                                        1.0 / HID_C, None, op0=Alu.mult)
                var = spool.tile([P, SBW], f32, tag="var")
                nc.vector.tensor_tensor(var[:, :nblk], m2n[:, :nblk],
                                        musq[:, :nblk], op=Alu.subtract)
                std = spool.tile([P, SBW], f32, tag="std")
                nc.scalar.activation(std[:, :nblk], var[:, :nblk], Act.Sqrt,
                                     bias=eps_c[:, :1])
                rstd = spool.tile([P, SBW], f32, tag="rstd")
                nc.vector.reciprocal(rstd[:, :nblk], std[:, :nblk])
                nmr = spool.tile([P, SBW], f32, tag="nmr")
                nc.vector.tensor_tensor(nmr[:, :nblk], mu[:, :nblk],
                                        rstd[:, :nblk], op=Alu.mult)
                nmrn = spool.tile([P, SBW], f32, tag="nmrn")
                nc.vector.tensor_scalar(nmrn[:, :nblk], nmr[:, :nblk],
                                        -1.0, None, op0=Alu.mult)
                hrelu = npool.tile([P, SBW * HID_C], fp16, tag="hrelu")
                for b in range(nblk):
                    hsl = slice(b * HID_C, (b + 1) * HID_C)
                    if not affine_ln:
                        nc.scalar.activation(hrelu[:, hsl], agg[:, hsl],
                                             Act.Relu,
                                             bias=nmrn[:, b:b + 1],
                                             scale=rstd[:, b:b + 1])
                    else:
                        nc.scalar.activation(hrelu[:, hsl], agg[:, hsl],
                                             Act.Copy,
                                             bias=0.0,
                                             scale=rstd[:, b:b + 1])
                if affine_ln:
                    # hrelu currently holds h*rstd; finish (x-mu)*rstd*g + b
                    # as ((h*rstd) + (-mu*rstd)) * gamma + beta, then relu.
                    sl = slice(0, nblk * HID_C)
                    t1 = npool.tile([P, SBW * HID_C], fp16, tag="at1")
                    nmr3 = nmrn[:, :nblk].unsqueeze(2).to_broadcast(
                        [P, nblk, HID_C])
                    nc.vector.tensor_tensor(
                        t1[:, sl].rearrange("p (b h) -> p b h", h=HID_C),
                        hrelu[:, sl].rearrange("p (b h) -> p b h", h=HID_C),
                        nmr3, op=Alu.add)
                    t2 = npool.tile([P, SBW * HID_C], fp16, tag="at2")
                    nc.vector.tensor_tensor(t2[:, sl], t1[:, sl],
                                            gam_sb[:, sl], op=Alu.mult)
                    t3 = npool.tile([P, SBW * HID_C], fp16, tag="at3")
                    nc.vector.tensor_tensor(t3[:, sl], t2[:, sl],
                                            bet_sb[:, sl], op=Alu.add)
                    nc.vector.tensor_scalar(hrelu[:, sl], t3[:, sl], 0.0,
                                            None, op0=Alu.max)
                st["hrelu"] = hrelu
                st["nblk"] = nblk
                sb_state[sb] = st

            def emit_node_b(sb):
                """Transpose + MLP head + output DMA for super-block sb."""
                st = sb_state.pop(sb)
                nblk = st["nblk"]
                hrelu = st.pop("hrelu")
                trp = tr_ps.tile([HID_C, SBW * P], fp16, tag="tr")
                for b in range(nblk):
                    nc.tensor.transpose(
                        trp[:, b * P:(b + 1) * P],
                        hrelu[:, b * HID_C:(b + 1) * HID_C], ident[:])
                hT = npool.tile([HID_C, SBW * P], fp16, tag="hT")
                nc.vector.tensor_copy(hT[:, :nblk * P], trp[:, :nblk * P])
                nch = (nblk + CW - 1) // CW
                for ch in range(nch):
                    cb = min(CW, nblk - ch * CW)
                    fps = mlp_ps.tile([OUT_C, CW * P], f32, tag="fT")
                    for b in range(cb):
                        bb = ch * CW + b
                        nc.tensor.matmul(fps[:, b * P:(b + 1) * P],
                                         lhsT=wlin_sb[:],
                                         rhs=hT[:, bb * P:(bb + 1) * P],
                                         start=True, stop=True)
                    fT = npool.tile([OUT_C, CW * P], fp16, tag="fTs")
                    nc.scalar.copy(fT[:, :cb * P], fps[:, :cb * P])
                    q1ps = mlp_ps.tile([MLP_H, CW * P], f32, tag="q1")
                    for b in range(cb):
                        nc.tensor.matmul(q1ps[:, b * P:(b + 1) * P],
                                         lhsT=wq1_sb[:],
                                         rhs=fT[:, b * P:(b + 1) * P],
                                         start=True, stop=True)
                    q1r = npool.tile([MLP_H, CW * P], fp16, tag="q1r")
                    nc.vector.tensor_scalar(q1r[:, :cb * P], q1ps[:, :cb * P],
                                            bq1_sb[:, :1], 0.0,
                                            op0=Alu.add, op1=Alu.max)
                    q2ps = mlp_ps.tile([N_ACT, CW * P], f32, tag="q2")
                    for b in range(cb):
                        nc.tensor.matmul(q2ps[:, b * P:(b + 1) * P],
                                         lhsT=wq2_sb[:],
                                         rhs=q1r[:, b * P:(b + 1) * P],
                                         start=True, stop=True)
                    qfin = npool.tile([N_ACT, CW * P], f32, tag="qfin")
                    nc.vector.tensor_scalar(qfin[:, :cb * P], q2ps[:, :cb * P],
                                            bq2_sb[:, :1], None, op0=Alu.add)
                    csl = slice((sb * SBW + ch * CW) * P,
                                (sb * SBW + ch * CW + cb) * P)
                    nc.sync.dma_start(qT.ap()[:, csl], qfin[:, :cb * P])

            # ---- main pipeline ----
            sb_done = -1          # last super-block whose A stage was emitted
            for s in range(ET + EDGE_LAG):
                if s < ET:
                    emit_front(s)
                t = s - EDGE_LAG
                if t < 0:
                    continue
                emit_scatter(t)
                k = int(tile_slot[t])
                kt = int(tile_kt[t])
                if kt == kts[k] - 1 and (k % SBW == SBW - 1 or k == NB - 1):
                    sb = k // SBW
                    emit_node_a(sb)
                    if sb >= 1:
                        emit_node_b(sb - 1)
                    sb_done = sb
            emit_node_b(sb_done)

    nc.compile()
    return nc, ET


def _get_program(kts: tuple, affine_ln: bool):
    key = (kts, affine_ln, RELU_PATTERN, MUL_PATTERN)
    if key not in _PROGRAM_CACHE:
        _PROGRAM_CACHE[key] = _build_program(kts, affine_ln)
    return _PROGRAM_CACHE[key]


def _prep_inputs(x, edge_src, edge_dst, edge_attr,
                 We, be, Wroot, bconv, gamma, beta,
                 Wlin, blin, Wq1, bq1, Wq2, bq2):
    """Host-side sharding: bucket+sort edges by destination block, order block
    slots by count, pad to shared per-slot capacities, build per-core input
    maps.  Index/layout work only."""
    f32 = np.float32
    x = np.asarray(x, f32)
    edge_src = np.asarray(edge_src)
    edge_dst = np.asarray(edge_dst)
    edge_attr = np.asarray(edge_attr, f32)

    order = np.argsort(edge_dst, kind="stable")
    dst_s = edge_dst[order]
    src_s = edge_src[order]
    attr_s = edge_attr[order]

    core_of = dst_s // NPC
    local = dst_s - core_of * NPC
    blk = local // P
    gblk = core_of * NB + blk
    counts = np.bincount(gblk, minlength=M * NB).reshape(M, NB)

    # order slots by per-core descending count; shared per-slot capacities
    perm = np.argsort(-counts, axis=1, kind="stable")      # [M, NB]
    sorted_counts = np.take_along_axis(counts, perm, axis=1)
    kts = np.maximum(1, -(-sorted_counts.max(axis=0) // P))  # [NB] tiles
    kts_t = tuple(int(v) for v in kts)
    offs = np.concatenate([[0], np.cumsum(kts)]).astype(np.int64) * P
    EPC = int(offs[-1])

    # slot index of each block per core
    slot_of_blk = np.empty((M, NB), np.int64)
    np.put_along_axis(slot_of_blk, perm, np.arange(NB)[None, :], axis=1)

    # position of each edge in its core's padded edge array
    slot = slot_of_blk[core_of, blk]                        # [E]
    starts = np.zeros(M * NB, np.int64)
    starts[1:] = np.cumsum(counts.reshape(-1))[:-1]
    rank = np.arange(E, dtype=np.int64) - starts[gblk]
    pos = offs[slot] + rank                                 # within core
    gpos = core_of.astype(np.int64) * EPC + pos

    tot = M * EPC
    attr_all = np.zeros((tot, EDGE_D + 1), np.float16)
    attr_all[gpos, :EDGE_D] = attr_s
    attr_all[gpos, EDGE_D] = 1.0
    xj_all = np.zeros((tot, IN_C), np.float16)
    xj_all[gpos] = x[src_s].astype(np.float16)
    oh_all = np.zeros((tot, P), np.float16)
    oh_all[gpos, local - blk * P] = 1.0

    attr_all = attr_all.reshape(M, EPC, EDGE_D + 1)
    xj_all = xj_all.reshape(M, EPC, IN_C)
    oh_all = oh_all.reshape(M, EPC, P)

    # node features per slot order, augmented with ones row
    x_pad = np.zeros((M, NPC_PAD, IN_C + 1), np.float16)
    for c in range(M):
        for k in range(NB):
            b = perm[c, k]
            lo = c * NPC + b * P
            nb_sz = min(P, NPC - b * P)
            x_pad[c, k * P:k * P + nb_sz, :IN_C] = x[lo:lo + nb_sz]
    x_pad[:, :, IN_C] = 1.0

    # parameters (replicated)
    We = np.asarray(We, f32)
    be = np.asarray(be, f32)
    Wroot = np.asarray(Wroot, f32)
    bconv = np.asarray(bconv, f32)
    gamma = np.asarray(gamma, f32)
    beta = np.asarray(beta, f32)
    Wlin = np.asarray(Wlin, f32)
    blin = np.asarray(blin, f32)
    Wq1 = np.asarray(Wq1, f32)
    bq1 = np.asarray(bq1, f32)
    Wq2 = np.asarray(Wq2, f32)
    bq2 = np.asarray(bq2, f32)

    affine_ln = not (np.all(gamma == 1.0) and np.all(beta == 0.0))

    weA = np.concatenate([We, be[None, :]], axis=0)            # [9, 512]
    # permute columns from (i, h) to (h, i) layout: the per-channel scatter
    # matmuls read prod[:, :, i] with h contiguous
    weA_perm = np.ascontiguousarray(
        weA.reshape(EDGE_D + 1, IN_C, HID_C).transpose(0, 2, 1)
           .reshape(EDGE_D + 1, IN_C * HID_C)).astype(np.float16)
    wrootA = np.concatenate([Wroot, bconv[None, :]], axis=0).astype(np.float16)
    bq1p = (blin @ Wq1 + bq1).astype(f32)                      # blin folded
    gam4 = np.broadcast_to(np.tile(gamma, SBW), (P, SBW * HID_C)).astype(
        np.float16).copy()
    bet4 = np.broadcast_to(np.tile(beta, SBW), (P, SBW * HID_C)).astype(
        np.float16).copy()

    in_maps = []
    for c in range(M):
        in_maps.append({
            "attrT": np.ascontiguousarray(attr_all[c].T),
            "xjg": np.ascontiguousarray(xj_all[c]),
            "ohg": np.ascontiguousarray(oh_all[c]),
            "xsT": np.ascontiguousarray(x_pad[c].T),
            "weA": weA_perm,
            "wrootA": wrootA,
            "wlin": Wlin.astype(np.float16),
            "wq1": Wq1.astype(np.float16),
            "wq2": Wq2.astype(np.float16),
            "bq1c": bq1p[:, None],
            "bq2c": bq2[:, None],
            "gamma4": gam4,
            "beta4": bet4,
        })
    return kts_t, affine_ln, perm, in_maps


def kernel(**inputs) -> np.ndarray:
    from concourse.bass_utils import run_bass_kernel_spmd

    kts_t, affine_ln, perm, in_maps = _prep_inputs(**inputs)
    nc, _ = _get_program(kts_t, affine_ln)
    res = run_bass_kernel_spmd(nc, in_maps, list(range(M)))
    q = np.empty((N, N_ACT), np.float32)
    for c in range(M):
        qTc = res.results[c]["qT"]
        for k in range(NB):
            b = int(perm[c, k])
            nb_sz = min(P, NPC - b * P)
            q[c * NPC + b * P: c * NPC + b * P + nb_sz] = \
                qTc[:, k * P:k * P + nb_sz].T
    return q


# revision 12
# speedup vs baseline: 1.3760x; 1.3760x over previous
"""Trainium2 Bass kernel for the DiscreteAgent GNN (NNConv + LN + MLP head).

Strategy (8 NeuronCores, SPMD, no collectives):
  * Edges bucketed by destination 128-node block; each core owns a disjoint
    6250-node range so outputs never overlap -> no all-reduce.
  * Within a core, block slots are ordered by descending edge count and the
    per-slot tile capacity is the max across cores, so the SPMD program is
    shared while padding stays small.
  * Per 128-edge tile: PE computes w_pre = [attr|1]^T @ [We;be] (512 cols),
    ACT/GPSIMD/DVE apply relu (split by tile for balance), DVE multiplies by
    the broadcast source features, and the i-contraction is FUSED into the
    scatter: 16 chained 32-col matmuls (one per input channel) accumulate
    onehot^T @ prod[:, :, i] into the block's [128, 32] PSUM slice.  The
    root-weight matmul opens each block's accumulation chain, so
    h = agg + x@Wroot + bconv materializes directly in PSUM.
  * One-hot scatter matrices are precomputed on the host and DMA'd (no
    per-tile is_equal on DVE).
  * Node phase per 4-block super-block: batched LN stats, per-block
    relu((h-mu)*rstd) fused into one ACT op (scale/bias per-partition),
    batched PE transpose, then the 3-layer MLP head in 2-block chunks with
    dual-op PSUM drains.
"""

import sys

import numpy as np

try:
    import concourse  # noqa: F401
except ImportError:  # pragma: no cover
    for _p in ("/opt/trn_rl_repo", "/opt/pypackages"):
        if _p not in sys.path:
            sys.path.insert(0, _p)

# ---- problem constants (hardcoded per contract) ----
N = 50000
E = 200000
IN_C = 16
HID_C = 32
EDGE_D = 8
OUT_C = 32
MLP_H = 128
N_ACT = 32

M = 8                 # cores
P = 128               # partitions
NPC = N // M          # 6250 nodes per core
NB = (NPC + P - 1) // P   # 49 block slots per core
NPC_PAD = NB * P      # 6272
G = 8                 # edge tiles per DMA group
SBW = 4               # blocks per node-phase super-block (stats/transpose)
CW = 2                # blocks per MLP chunk
EDGE_LAG = 4          # scatter trails the relu/mul front by this many tiles

# relu engine per tile, cycled: 'A'=ACT, 'V'=DVE (GPSIMD cannot read PSUM)
RELU_PATTERN = "AAAAAVAAAAAV"
# mul engine per tile: 'V'=DVE, 'G'=GPSIMD
MUL_PATTERN = "GVGVV"

_PROGRAM_CACHE: dict = {}


def _build_program(kts: tuple, affine_ln: bool):
    """Build + compile the SPMD Bass program.

    kts: per-block-slot edge-tile counts (len NB, each >= 1).
    affine_ln: if True, gamma/beta are non-trivial and applied explicitly.
    """
    import concourse.tile as tile
    from concourse import bacc, mybir
    from concourse.masks import make_identity

    f32 = mybir.dt.float32
    fp16 = mybir.dt.float16
    Act = mybir.ActivationFunctionType
    Alu = mybir.AluOpType

    ET = int(sum(kts))            # edge tiles per core
    EPC = ET * P                  # padded edge slots per core
    offs = np.concatenate([[0], np.cumsum(kts)]).astype(int)  # tile offsets

    # flat tile -> (slot, kt) map
    tile_slot = np.empty(ET, int)
    tile_kt = np.empty(ET, int)
    for k in range(NB):
        tile_slot[offs[k]:offs[k + 1]] = k
        tile_kt[offs[k]:offs[k + 1]] = np.arange(kts[k])

    nc = bacc.Bacc("TRN2", target_bir_lowering=False, debug=False, num_devices=M)

    # --- DRAM I/O (per core) ---
    attrT = nc.dram_tensor("attrT", [EDGE_D + 1, EPC], fp16, kind="ExternalInput")
    xjg = nc.dram_tensor("xjg", [EPC, IN_C], fp16, kind="ExternalInput")
    ohg = nc.dram_tensor("ohg", [EPC, P], fp16, kind="ExternalInput")
    xsT = nc.dram_tensor("xsT", [IN_C + 1, NPC_PAD], fp16, kind="ExternalInput")
    weA = nc.dram_tensor("weA", [EDGE_D + 1, IN_C * HID_C], fp16, kind="ExternalInput")
    wrootA = nc.dram_tensor("wrootA", [IN_C + 1, HID_C], fp16, kind="ExternalInput")
    wlin = nc.dram_tensor("wlin", [HID_C, OUT_C], fp16, kind="ExternalInput")
    wq1 = nc.dram_tensor("wq1", [OUT_C, MLP_H], fp16, kind="ExternalInput")
    wq2 = nc.dram_tensor("wq2", [MLP_H, N_ACT], fp16, kind="ExternalInput")
    bq1c = nc.dram_tensor("bq1c", [MLP_H, 1], f32, kind="ExternalInput")
    bq2c = nc.dram_tensor("bq2c", [N_ACT, 1], f32, kind="ExternalInput")
    gamma4 = nc.dram_tensor("gamma4", [P, SBW * HID_C], fp16, kind="ExternalInput")
    beta4 = nc.dram_tensor("beta4", [P, SBW * HID_C], fp16, kind="ExternalInput")
    qT = nc.dram_tensor("qT", [N_ACT, NPC_PAD], f32, kind="ExternalOutput")

    NSB = (NB + SBW - 1) // SBW   # super-blocks

    with tile.TileContext(nc) as tc:
        with (
            tc.tile_pool(name="const", bufs=1) as cpool,
            tc.tile_pool(name="edge_in", bufs=3) as epool,
            tc.tile_pool(name="wrelu", bufs=6) as wpool,
            tc.tile_pool(name="node", bufs=2) as npool,
            tc.tile_pool(name="stats", bufs=2) as spool,
            tc.tile_pool(name="wpre_ps", bufs=2, space="PSUM") as wpre_ps,
            tc.tile_pool(name="agg_ps", bufs=2, space="PSUM") as agg_ps,
            tc.tile_pool(name="root_ps", bufs=1, space="PSUM") as root_ps,
            tc.tile_pool(name="tr_ps", bufs=1, space="PSUM") as tr_ps,
            tc.tile_pool(name="mlp_ps", bufs=2, space="PSUM") as mlp_ps,
        ):
            group_state = {}
            NGROUPS = (ET + G - 1) // G

            def emit_group_load(g):
                gs = min(G, ET - g * G)
                esl = slice(g * G * P, (g * G + gs) * P)
                attr_g = epool.tile([EDGE_D + 1, G * P], fp16, tag="attr")
                nc.sync.dma_start(attr_g[:, :gs * P], attrT.ap()[:, esl])
                xj_g = epool.tile([P, G, IN_C], fp16, tag="xj")
                nc.sync.dma_start(
                    xj_g[:, :gs, :],
                    xjg.ap()[esl, :].rearrange("(tt p) i -> p tt i", p=P))
                oh_g = epool.tile([P, G, P], fp16, tag="oh")
                nc.sync.dma_start(
                    oh_g[:, :gs, :],
                    ohg.ap()[esl, :].rearrange("(tt p) n -> p tt n", p=P))
                group_state[g] = (attr_g, xj_g, oh_g)

            for g in range(2):
                emit_group_load(g)

            # ---- persistent constants in SBUF ----
            we_sb = cpool.tile([EDGE_D + 1, IN_C * HID_C], fp16, tag="we")
            nc.sync.dma_start(we_sb[:], weA.ap()[:])
            xsT_sb = cpool.tile([IN_C + 1, NPC_PAD], fp16, tag="xsT")
            nc.sync.dma_start(xsT_sb[:], xsT.ap()[:])
            wroot_sb = cpool.tile([IN_C + 1, HID_C], fp16, tag="wroot")
            nc.sync.dma_start(wroot_sb[:], wrootA.ap()[:])
            wlin_sb = cpool.tile([HID_C, OUT_C], fp16, tag="wlin")
            nc.sync.dma_start(wlin_sb[:], wlin.ap()[:])
            wq1_sb = cpool.tile([OUT_C, MLP_H], fp16, tag="wq1")
            nc.sync.dma_start(wq1_sb[:], wq1.ap()[:])
            wq2_sb = cpool.tile([MLP_H, N_ACT], fp16, tag="wq2")
            nc.sync.dma_start(wq2_sb[:], wq2.ap()[:])
            bq1_sb = cpool.tile([MLP_H, 1], f32, tag="bq1")
            nc.sync.dma_start(bq1_sb[:], bq1c.ap()[:])
            bq2_sb = cpool.tile([N_ACT, 1], f32, tag="bq2")
            nc.sync.dma_start(bq2_sb[:], bq2c.ap()[:])
            if affine_ln:
                gam_sb = cpool.tile([P, SBW * HID_C], fp16, tag="gam")
                nc.sync.dma_start(gam_sb[:], gamma4.ap()[:])
                bet_sb = cpool.tile([P, SBW * HID_C], fp16, tag="bet")
                nc.sync.dma_start(bet_sb[:], beta4.ap()[:])

            ident = cpool.tile([P, P], fp16, tag="ident")
            make_identity(nc, ident[:])
            eps_c = cpool.tile([P, 1], f32, tag="eps")
            nc.gpsimd.memset(eps_c[:], 1e-5)

            # ---- pipeline state ----
            edge_state = {}       # t -> prod tile
            agg_by_blk = {}       # k -> expanded agg psum tile [P, 512]
            root_by_sb = {}       # sb -> root psum tile [P, SBW*HID_C]
            sbt = {}              # sb -> dict of stat tiles (filled per block)
            sb_state = {}         # sb -> dict (stage A outputs)

            def emit_front(t):
                g, tt = divmod(t, G)
                if tt == 0 and g not in group_state:
                    emit_group_load(g)
                if tt == 0 and g + 1 < NGROUPS and g + 1 not in group_state:
                    emit_group_load(g + 1)
                attr_g, xj_g, oh_g = group_state[g]

                wpre = wpre_ps.tile([P, IN_C * HID_C], f32, tag="wpre")
                nc.tensor.matmul(wpre[:], lhsT=attr_g[:, tt * P:(tt + 1) * P],
                                 rhs=we_sb[:], start=True, stop=True)
                wrelu = wpool.tile([P, IN_C * HID_C], fp16, tag="wrelu")
                eng = RELU_PATTERN[t % len(RELU_PATTERN)]
                if eng == "A":
                    nc.scalar.activation(wrelu[:], wpre[:], Act.Relu)
                else:
                    nc.vector.tensor_scalar(wrelu[:], wpre[:], 0.0, None,
                                            op0=Alu.max)
                prod = wpool.tile([P, IN_C * HID_C], fp16, tag="prod")
                xj_b = xj_g[:, t % G, :].unsqueeze(1).to_broadcast(
                    [P, HID_C, IN_C])
                prod_3d = prod[:].rearrange("p (h i) -> p h i", h=HID_C)
                wrelu_3d = wrelu[:].rearrange("p (h i) -> p h i", h=HID_C)
                if MUL_PATTERN[t % len(MUL_PATTERN)] == "G":
                    nc.gpsimd.tensor_tensor(prod_3d, wrelu_3d, xj_b,
                                            op=Alu.mult)
                else:
                    nc.vector.tensor_tensor(prod_3d, wrelu_3d, xj_b,
                                            op=Alu.mult)
                edge_state[t] = (prod, g, tt)

            def emit_scatter(t):
                k = int(tile_slot[t])
                kt = int(tile_kt[t])
                sb, kb = divmod(k, SBW)
                prod, g, tt = edge_state.pop(t)
                oh_g = group_state[g][2]
                if kb == 0 and kt == 0:
                    root_by_sb[sb] = root_ps.tile([P, SBW * HID_C], f32,
                                                  tag="root", name="root")
                if kt == 0:
                    agg_by_blk[k] = agg_ps.tile([P, IN_C * HID_C], f32,
                                                tag="agg", name="agg")
                    nsl = slice(k * P, (k + 1) * P)
                    nc.tensor.matmul(
                        root_by_sb[sb][:, kb * HID_C:(kb + 1) * HID_C],
                        lhsT=xsT_sb[:, nsl], rhs=wroot_sb[:],
                        start=True, stop=True)
                nc.tensor.matmul(agg_by_blk[k][:], lhsT=oh_g[:, tt, :],
                                 rhs=prod[:],
                                 start=(kt == 0), stop=(kt == kts[k] - 1))

            def emit_block_end(k):
                """Right after block k's last scatter: expanded i-reduce and
                fused h-add + LN stats, freeing the agg psum quickly."""
                sb, b = divmod(k, SBW)
                if b == 0:
                    h_sb = npool.tile([P, SBW * HID_C], f32, tag="h_sb",
                                      name="h_sb")
                    red = npool.tile([P, SBW * HID_C], f32, tag="red",
                                     name="red")
                    musum = spool.tile([P, SBW], f32, tag="musum",
                                       name="musum")
                    m2 = spool.tile([P, SBW], f32, tag="m2", name="m2")
                    hsq = wpool.tile([P, SBW * HID_C], fp16, tag="hsq",
                                     name="hsq")
                    sbt[sb] = {"h_sb": h_sb, "red": red, "musum": musum,
                               "m2": m2, "hsq": hsq}
                st = sbt[sb]
                root = root_by_sb[sb]
                agg = agg_by_blk.pop(k)
                hsl = slice(b * HID_C, (b + 1) * HID_C)
                nc.vector.tensor_reduce(
                    st["red"][:, hsl],
                    agg[:].rearrange("p (h i) -> p h i", h=HID_C),
                    axis=mybir.AxisListType.X, op=Alu.add)
                nc.vector.tensor_add(st["h_sb"][:, hsl], st["red"][:, hsl],
                                     root[:, hsl])
                nc.scalar.activation(st["hsq"][:, hsl], st["h_sb"][:, hsl],
                                     Act.Square,
                                     accum_out=st["m2"][:, b:b + 1])

            def emit_node_a(sb):
                """Batched LN scalar chain + fused relu((h-mu)*rstd)."""
                nblk = min(SBW, NB - sb * SBW)
                root_by_sb.pop(sb)
                stt = sbt.pop(sb)
                st = {}
                h_sb = stt["h_sb"]
                musum = stt["musum"]
                m2 = stt["m2"]
                nc.vector.tensor_reduce(
                    musum[:, :nblk],
                    h_sb[:, :nblk * HID_C].rearrange("p (b h) -> p b h",
                                                     h=HID_C),
                    axis=mybir.AxisListType.X, op=Alu.add)
                mu = spool.tile([P, SBW], f32, tag="mu")
                nc.vector.tensor_scalar(mu[:, :nblk], musum[:, :nblk],
                                        1.0 / HID_C, None, op0=Alu.mult)
                musq = spool.tile([P, SBW], f32, tag="musq")
                nc.vector.tensor_tensor(musq[:, :nblk], mu[:, :nblk],
                                        mu[:, :nblk], op=Alu.mult)
                m2n = spool.tile([P, SBW], f32, tag="m2n")
                nc.vector.tensor_scalar(m2n[:, :nblk], m2[:, :nblk],
                                        1.0 / HID_C, None, op0=Alu.mult)
                var = spool.tile([P, SBW], f32, tag="var")
                nc.vector.tensor_tensor(var[:, :nblk], m2n[:, :nblk],
                                        musq[:, :nblk], op=Alu.subtract)
                std = spool.tile([P, SBW], f32, tag="std")
                nc.scalar.activation(std[:, :nblk], var[:, :nblk], Act.Sqrt,
                                     bias=eps_c[:, :1])
                rstd = spool.tile([P, SBW], f32, tag="rstd")
                nc.vector.reciprocal(rstd[:, :nblk], std[:, :nblk])
                nmr = spool.tile([P, SBW], f32, tag="nmr")
                nc.vector.tensor_tensor(nmr[:, :nblk], mu[:, :nblk],
                                        rstd[:, :nblk], op=Alu.mult)
                nmrn = spool.tile([P, SBW], f32, tag="nmrn")
                nc.vector.tensor_scalar(nmrn[:, :nblk], nmr[:, :nblk],
                                        -1.0, None, op0=Alu.mult)
                hrelu = npool.tile([P, SBW * HID_C], fp16, tag="hrelu")
                for b in range(nblk):
                    hsl = slice(b * HID_C, (b + 1) * HID_C)
                    if not affine_ln:
                        nc.scalar.activation(hrelu[:, hsl], h_sb[:, hsl],
                                             Act.Relu,
                                             bias=nmrn[:, b:b + 1],
                                             scale=rstd[:, b:b + 1])
                    else:
                        nc.scalar.activation(hrelu[:, hsl], h_sb[:, hsl],
                                             Act.Copy,
                                             bias=0.0,
                                             scale=rstd[:, b:b + 1])
                if affine_ln:
                    # hrelu currently holds h*rstd; finish (x-mu)*rstd*g + b
                    # as ((h*rstd) + (-mu*rstd)) * gamma + beta, then relu.
                    sl = slice(0, nblk * HID_C)
                    t1 = npool.tile([P, SBW * HID_C], fp16, tag="at1")
                    nmr3 = nmrn[:, :nblk].unsqueeze(2).to_broadcast(
                        [P, nblk, HID_C])
                    nc.vector.tensor_tensor(
                        t1[:, sl].rearrange("p (b h) -> p b h", h=HID_C),
                        hrelu[:, sl].rearrange("p (b h) -> p b h", h=HID_C),
                        nmr3, op=Alu.add)
                    t2 = npool.tile([P, SBW * HID_C], fp16, tag="at2")
                    nc.vector.tensor_tensor(t2[:, sl], t1[:, sl],
                                            gam_sb[:, sl], op=Alu.mult)
                    t3 = npool.tile([P, SBW * HID_C], fp16, tag="at3")
                    nc.vector.tensor_tensor(t3[:, sl], t2[:, sl],
                                            bet_sb[:, sl], op=Alu.add)
                    nc.vector.tensor_scalar(hrelu[:, sl], t3[:, sl], 0.0,
                                            None, op0=Alu.max)
                st["hrelu"] = hrelu
                st["nblk"] = nblk
                sb_state[sb] = st

            def emit_node_b(sb):
                """Transpose + MLP head + output DMA for super-block sb."""
                st = sb_state.pop(sb)
                nblk = st["nblk"]
                hrelu = st.pop("hrelu")
                trp = tr_ps.tile([HID_C, SBW * P], fp16, tag="tr")
                for b in range(nblk):
                    nc.tensor.transpose(
                        trp[:, b * P:(b + 1) * P],
                        hrelu[:, b * HID_C:(b + 1) * HID_C], ident[:])
                hT = npool.tile([HID_C, SBW * P], fp16, tag="hT")
                nc.vector.tensor_copy(hT[:, :nblk * P], trp[:, :nblk * P])
                nch = (nblk + CW - 1) // CW
                for ch in range(nch):
                    cb = min(CW, nblk - ch * CW)
                    fps = mlp_ps.tile([OUT_C, CW * P], f32, tag="mlp",
                                      name="fps")
                    for b in range(cb):
                        bb = ch * CW + b
                        nc.tensor.matmul(fps[:, b * P:(b + 1) * P],
                                         lhsT=wlin_sb[:],
                                         rhs=hT[:, bb * P:(bb + 1) * P],
                                         start=True, stop=True)
                    fT = npool.tile([OUT_C, CW * P], fp16, tag="fTs")
                    nc.scalar.copy(fT[:, :cb * P], fps[:, :cb * P])
                    q1ps = mlp_ps.tile([MLP_H, CW * P], f32, tag="mlp",
                                       name="q1ps")
                    for b in range(cb):
                        nc.tensor.matmul(q1ps[:, b * P:(b + 1) * P],
                                         lhsT=wq1_sb[:],
                                         rhs=fT[:, b * P:(b + 1) * P],
                                         start=True, stop=True)
                    q1r = npool.tile([MLP_H, CW * P], fp16, tag="q1r")
                    nc.vector.tensor_scalar(q1r[:, :cb * P], q1ps[:, :cb * P],
                                            bq1_sb[:, :1], 0.0,
                                            op0=Alu.add, op1=Alu.max)
                    q2ps = mlp_ps.tile([N_ACT, CW * P], f32, tag="mlp",
                                        name="q2ps")
                    for b in range(cb):
                        nc.tensor.matmul(q2ps[:, b * P:(b + 1) * P],
                                         lhsT=wq2_sb[:],
                                         rhs=q1r[:, b * P:(b + 1) * P],
                                         start=True, stop=True)
                    qfin = npool.tile([N_ACT, CW * P], f32, tag="qfin")
                    nc.vector.tensor_scalar(qfin[:, :cb * P], q2ps[:, :cb * P],
                                            bq2_sb[:, :1], None, op0=Alu.add)
                    csl = slice((sb * SBW + ch * CW) * P,
                                (sb * SBW + ch * CW + cb) * P)
                    nc.sync.dma_start(qT.ap()[:, csl], qfin[:, :cb * P])

            # ---- main pipeline ----
            sb_done = -1          # last super-block whose A stage was emitted
            for s in range(ET + EDGE_LAG):
                if s < ET:
                    emit_front(s)
                t = s - EDGE_LAG
                if t < 0:
                    continue
                emit_scatter(t)
                k = int(tile_slot[t])
                kt = int(tile_kt[t])
                if kt == kts[k] - 1:
                    emit_block_end(k)
                    if k % SBW == SBW - 1 or k == NB - 1:
                        sb = k // SBW
                        emit_node_a(sb)
                        if sb >= 1:
                            emit_node_b(sb - 1)
                        sb_done = sb
            emit_node_b(sb_done)

    nc.compile()
    return nc, ET


def _get_program(kts: tuple, affine_ln: bool):
    key = (kts, affine_ln, RELU_PATTERN, MUL_PATTERN)
    if key not in _PROGRAM_CACHE:
        _PROGRAM_CACHE[key] = _build_program(kts, affine_ln)
    return _PROGRAM_CACHE[key]


def _prep_inputs(x, edge_src, edge_dst, edge_attr,
                 We, be, Wroot, bconv, gamma, beta,
                 Wlin, blin, Wq1, bq1, Wq2, bq2):
    """Host-side sharding: bucket+sort edges by destination block, order block
    slots by count, pad to shared per-slot capacities, build per-core input
    maps.  Index/layout work only."""
    f32 = np.float32
    x = np.asarray(x, f32)
    edge_src = np.asarray(edge_src)
    edge_dst = np.asarray(edge_dst)
    edge_attr = np.asarray(edge_attr, f32)

    order = np.argsort(edge_dst, kind="stable")
    dst_s = edge_dst[order]
    src_s = edge_src[order]
    attr_s = edge_attr[order]

    core_of = dst_s // NPC
    local = dst_s - core_of * NPC
    blk = local // P
    gblk = core_of * NB + blk
    counts = np.bincount(gblk, minlength=M * NB).reshape(M, NB)

    # order slots by per-core descending count; shared per-slot capacities
    perm = np.argsort(-counts, axis=1, kind="stable")      # [M, NB]
    sorted_counts = np.take_along_axis(counts, perm, axis=1)
    kts = np.maximum(1, -(-sorted_counts.max(axis=0) // P))  # [NB] tiles
    kts_t = tuple(int(v) for v in kts)
    offs = np.concatenate([[0], np.cumsum(kts)]).astype(np.int64) * P
    EPC = int(offs[-1])

    # slot index of each block per core
    slot_of_blk = np.empty((M, NB), np.int64)
    np.put_along_axis(slot_of_blk, perm, np.arange(NB)[None, :], axis=1)

    # position of each edge in its core's padded edge array
    slot = slot_of_blk[core_of, blk]                        # [E]
    starts = np.zeros(M * NB, np.int64)
    starts[1:] = np.cumsum(counts.reshape(-1))[:-1]
    rank = np.arange(E, dtype=np.int64) - starts[gblk]
    pos = offs[slot] + rank                                 # within core
    gpos = core_of.astype(np.int64) * EPC + pos

    tot = M * EPC
    attr_all = np.zeros((tot, EDGE_D + 1), np.float16)
    attr_all[gpos, :EDGE_D] = attr_s
    attr_all[gpos, EDGE_D] = 1.0
    xj_all = np.zeros((tot, IN_C), np.float16)
    xj_all[gpos] = x[src_s].astype(np.float16)
    oh_all = np.zeros((tot, P), np.float16)
    oh_all[gpos, local - blk * P] = 1.0

    attr_all = attr_all.reshape(M, EPC, EDGE_D + 1)
    xj_all = xj_all.reshape(M, EPC, IN_C)
    oh_all = oh_all.reshape(M, EPC, P)

    # node features per slot order, augmented with ones row
    x_pad = np.zeros((M, NPC_PAD, IN_C + 1), np.float16)
    for c in range(M):
        for k in range(NB):
            b = perm[c, k]
            lo = c * NPC + b * P
            nb_sz = min(P, NPC - b * P)
            x_pad[c, k * P:k * P + nb_sz, :IN_C] = x[lo:lo + nb_sz]
    x_pad[:, :, IN_C] = 1.0

    # parameters (replicated)
    We = np.asarray(We, f32)
    be = np.asarray(be, f32)
    Wroot = np.asarray(Wroot, f32)
    bconv = np.asarray(bconv, f32)
    gamma = np.asarray(gamma, f32)
    beta = np.asarray(beta, f32)
    Wlin = np.asarray(Wlin, f32)
    blin = np.asarray(blin, f32)
    Wq1 = np.asarray(Wq1, f32)
    bq1 = np.asarray(bq1, f32)
    Wq2 = np.asarray(Wq2, f32)
    bq2 = np.asarray(bq2, f32)

    affine_ln = not (np.all(gamma == 1.0) and np.all(beta == 0.0))

    weA = np.concatenate([We, be[None, :]], axis=0)            # [9, 512]
    # permute columns from (i, h) to (h, i) layout: the per-channel scatter
    # matmuls read prod[:, :, i] with h contiguous
    weA_perm = np.ascontiguousarray(
        weA.reshape(EDGE_D + 1, IN_C, HID_C).transpose(0, 2, 1)
           .reshape(EDGE_D + 1, IN_C * HID_C)).astype(np.float16)
    wrootA = np.concatenate([Wroot, bconv[None, :]], axis=0).astype(np.float16)
    bq1p = (blin @ Wq1 + bq1).astype(f32)                      # blin folded
    gam4 = np.broadcast_to(np.tile(gamma, SBW), (P, SBW * HID_C)).astype(
        np.float16).copy()
    bet4 = np.broadcast_to(np.tile(beta, SBW), (P, SBW * HID_C)).astype(
        np.float16).copy()

    in_maps = []
    for c in range(M):
        in_maps.append({
            "attrT": np.ascontiguousarray(attr_all[c].T),
            "xjg": np.ascontiguousarray(xj_all[c]),
            "ohg": np.ascontiguousarray(oh_all[c]),
            "xsT": np.ascontiguousarray(x_pad[c].T),
            "weA": weA_perm,
            "wrootA": wrootA,
            "wlin": Wlin.astype(np.float16),
            "wq1": Wq1.astype(np.float16),
            "wq2": Wq2.astype(np.float16),
            "bq1c": bq1p[:, None],
            "bq2c": bq2[:, None],
            "gamma4": gam4,
            "beta4": bet4,
        })
    return kts_t, affine_ln, perm, in_maps


def kernel(**inputs) -> np.ndarray:
    from concourse.bass_utils import run_bass_kernel_spmd

    kts_t, affine_ln, perm, in_maps = _prep_inputs(**inputs)
    nc, _ = _get_program(kts_t, affine_ln)
    res = run_bass_kernel_spmd(nc, in_maps, list(range(M)))
    q = np.empty((N, N_ACT), np.float32)
    for c in range(M):
        qTc = res.results[c]["qT"]
        for k in range(NB):
            b = int(perm[c, k])
            nb_sz = min(P, NPC - b * P)
            q[c * NPC + b * P: c * NPC + b * P + nb_sz] = \
                qTc[:, k * P:k * P + nb_sz].T
    return q


# revision 13
# speedup vs baseline: 1.6315x; 1.1857x over previous
"""Trainium2 Bass kernel for the DiscreteAgent GNN (NNConv + LN + MLP head).

Strategy (8 NeuronCores, SPMD, no collectives):
  * Edges bucketed by destination 128-node block; each core owns a disjoint
    6250-node range so outputs never overlap -> no all-reduce.
  * Within a core, block slots are ordered by descending edge count and the
    per-slot tile capacity is the max across cores, so the SPMD program is
    shared while padding stays small.
  * Per 128-edge tile: PE computes w_pre = [attr|1]^T @ [We;be] (512 cols),
    ACT/GPSIMD/DVE apply relu (split by tile for balance), DVE multiplies by
    the broadcast source features, and the i-contraction is FUSED into the
    scatter: 16 chained 32-col matmuls (one per input channel) accumulate
    onehot^T @ prod[:, :, i] into the block's [128, 32] PSUM slice.  The
    root-weight matmul opens each block's accumulation chain, so
    h = agg + x@Wroot + bconv materializes directly in PSUM.
  * One-hot scatter matrices are precomputed on the host and DMA'd (no
    per-tile is_equal on DVE).
  * Node phase per 4-block super-block: batched LN stats, per-block
    relu((h-mu)*rstd) fused into one ACT op (scale/bias per-partition),
    batched PE transpose, then the 3-layer MLP head in 2-block chunks with
    dual-op PSUM drains.
"""

import sys

import numpy as np

try:
    import concourse  # noqa: F401
except ImportError:  # pragma: no cover
    for _p in ("/opt/trn_rl_repo", "/opt/pypackages"):
        if _p not in sys.path:
            sys.path.insert(0, _p)

# ---- problem constants (hardcoded per contract) ----
N = 50000
E = 200000
IN_C = 16
HID_C = 32
EDGE_D = 8
OUT_C = 32
MLP_H = 128
N_ACT = 32

M = 8                 # cores
P = 128               # partitions
AK = 128              # contraction rows for the w_pre matmul (zero-padded:
                      # K=9 matmuls stream at half rate on HW)
NPC = N // M          # 6250 nodes per core
NB = (NPC + P - 1) // P   # 49 block slots per core
NPC_PAD = NB * P      # 6272
G = 8                 # edge tiles per DMA group
SBW = 4               # blocks per node-phase super-block (stats/transpose)
CW = 2                # blocks per MLP chunk
EDGE_LAG = 4          # scatter trails the relu/mul front by this many tiles

# relu engine per tile, cycled: 'A'=ACT, 'V'=DVE (GPSIMD cannot read PSUM)
RELU_PATTERN = "AAAAAVAAAAAV"
# mul engine per tile: 'V'=DVE, 'G'=GPSIMD
MUL_PATTERN = "GVGVV"

_PROGRAM_CACHE: dict = {}


def _build_program(kts: tuple, affine_ln: bool):
    """Build + compile the SPMD Bass program.

    kts: per-block-slot edge-tile counts (len NB, each >= 1).
    affine_ln: if True, gamma/beta are non-trivial and applied explicitly.
    """
    import concourse.tile as tile
    from concourse import bacc, mybir
    from concourse.masks import make_identity

    f32 = mybir.dt.float32
    fp16 = mybir.dt.float16
    Act = mybir.ActivationFunctionType
    Alu = mybir.AluOpType

    ET = int(sum(kts))            # edge tiles per core
    EPC = ET * P                  # padded edge slots per core
    offs = np.concatenate([[0], np.cumsum(kts)]).astype(int)  # tile offsets

    # flat tile -> (slot, kt) map
    tile_slot = np.empty(ET, int)
    tile_kt = np.empty(ET, int)
    for k in range(NB):
        tile_slot[offs[k]:offs[k + 1]] = k
        tile_kt[offs[k]:offs[k + 1]] = np.arange(kts[k])

    nc = bacc.Bacc("TRN2", target_bir_lowering=False, debug=False, num_devices=M)

    # --- DRAM I/O (per core) ---
    attrT = nc.dram_tensor("attrT", [AK, EPC], fp16, kind="ExternalInput")
    xjg = nc.dram_tensor("xjg", [EPC, IN_C], fp16, kind="ExternalInput")
    ohg = nc.dram_tensor("ohg", [EPC, P], fp16, kind="ExternalInput")
    xsT = nc.dram_tensor("xsT", [IN_C + 1, NPC_PAD], fp16, kind="ExternalInput")
    weA = nc.dram_tensor("weA", [AK, IN_C * HID_C], fp16, kind="ExternalInput")
    wrootA = nc.dram_tensor("wrootA", [IN_C + 1, HID_C], fp16, kind="ExternalInput")
    wlin = nc.dram_tensor("wlin", [HID_C, OUT_C], fp16, kind="ExternalInput")
    wq1 = nc.dram_tensor("wq1", [OUT_C, MLP_H], fp16, kind="ExternalInput")
    wq2 = nc.dram_tensor("wq2", [MLP_H, N_ACT], fp16, kind="ExternalInput")
    bq1c = nc.dram_tensor("bq1c", [MLP_H, 1], f32, kind="ExternalInput")
    bq2c = nc.dram_tensor("bq2c", [N_ACT, 1], f32, kind="ExternalInput")
    gamma4 = nc.dram_tensor("gamma4", [P, SBW * HID_C], fp16, kind="ExternalInput")
    beta4 = nc.dram_tensor("beta4", [P, SBW * HID_C], fp16, kind="ExternalInput")
    qT = nc.dram_tensor("qT", [N_ACT, NPC_PAD], f32, kind="ExternalOutput")

    NSB = (NB + SBW - 1) // SBW   # super-blocks

    with tile.TileContext(nc) as tc:
        with (
            tc.tile_pool(name="const", bufs=1) as cpool,
            tc.tile_pool(name="edge_in", bufs=3) as epool,
            tc.tile_pool(name="wrelu", bufs=6) as wpool,
            tc.tile_pool(name="node", bufs=2) as npool,
            tc.tile_pool(name="stats", bufs=2) as spool,
            tc.tile_pool(name="wpre_ps", bufs=2, space="PSUM") as wpre_ps,
            tc.tile_pool(name="agg_ps", bufs=2, space="PSUM") as agg_ps,
            tc.tile_pool(name="root_ps", bufs=1, space="PSUM") as root_ps,
            tc.tile_pool(name="tr_ps", bufs=1, space="PSUM") as tr_ps,
            tc.tile_pool(name="mlp_ps", bufs=2, space="PSUM") as mlp_ps,
        ):
            group_state = {}
            NGROUPS = (ET + G - 1) // G

            def emit_group_load(g):
                gs = min(G, ET - g * G)
                esl = slice(g * G * P, (g * G + gs) * P)
                attr_g = epool.tile([AK, G * P], fp16, tag="attr")
                nc.sync.dma_start(attr_g[:, :gs * P], attrT.ap()[:, esl])
                xj_g = epool.tile([P, G, IN_C], fp16, tag="xj")
                nc.sync.dma_start(
                    xj_g[:, :gs, :],
                    xjg.ap()[esl, :].rearrange("(tt p) i -> p tt i", p=P))
                oh_g = epool.tile([P, G, P], fp16, tag="oh")
                nc.sync.dma_start(
                    oh_g[:, :gs, :],
                    ohg.ap()[esl, :].rearrange("(tt p) n -> p tt n", p=P))
                group_state[g] = (attr_g, xj_g, oh_g)

            for g in range(2):
                emit_group_load(g)

            # ---- persistent constants in SBUF ----
            we_sb = cpool.tile([AK, IN_C * HID_C], fp16, tag="we")
            nc.sync.dma_start(we_sb[:], weA.ap()[:])
            xsT_sb = cpool.tile([IN_C + 1, NPC_PAD], fp16, tag="xsT")
            nc.sync.dma_start(xsT_sb[:], xsT.ap()[:])
            wroot_sb = cpool.tile([IN_C + 1, HID_C], fp16, tag="wroot")
            nc.sync.dma_start(wroot_sb[:], wrootA.ap()[:])
            wlin_sb = cpool.tile([HID_C, OUT_C], fp16, tag="wlin")
            nc.sync.dma_start(wlin_sb[:], wlin.ap()[:])
            wq1_sb = cpool.tile([OUT_C, MLP_H], fp16, tag="wq1")
            nc.sync.dma_start(wq1_sb[:], wq1.ap()[:])
            wq2_sb = cpool.tile([MLP_H, N_ACT], fp16, tag="wq2")
            nc.sync.dma_start(wq2_sb[:], wq2.ap()[:])
            bq1_sb = cpool.tile([MLP_H, 1], f32, tag="bq1")
            nc.sync.dma_start(bq1_sb[:], bq1c.ap()[:])
            bq2_sb = cpool.tile([N_ACT, 1], f32, tag="bq2")
            nc.sync.dma_start(bq2_sb[:], bq2c.ap()[:])
            if affine_ln:
                gam_sb = cpool.tile([P, SBW * HID_C], fp16, tag="gam")
                nc.sync.dma_start(gam_sb[:], gamma4.ap()[:])
                bet_sb = cpool.tile([P, SBW * HID_C], fp16, tag="bet")
                nc.sync.dma_start(bet_sb[:], beta4.ap()[:])

            ident = cpool.tile([P, P], fp16, tag="ident")
            make_identity(nc, ident[:])
            eps_c = cpool.tile([P, 1], f32, tag="eps")
            nc.gpsimd.memset(eps_c[:], 1e-5)

            # ---- pipeline state ----
            edge_state = {}       # t -> prod tile
            agg_by_blk = {}       # k -> expanded agg psum tile [P, 512]
            root_by_sb = {}       # sb -> root psum tile [P, SBW*HID_C]
            sbt = {}              # sb -> dict of stat tiles (filled per block)
            sb_state = {}         # sb -> dict (stage A outputs)

            def emit_front(t):
                g, tt = divmod(t, G)
                if tt == 0 and g not in group_state:
                    emit_group_load(g)
                if tt == 0 and g + 1 < NGROUPS and g + 1 not in group_state:
                    emit_group_load(g + 1)
                attr_g, xj_g, oh_g = group_state[g]

                wpre = wpre_ps.tile([P, IN_C * HID_C], f32, tag="wpre")
                nc.tensor.matmul(wpre[:], lhsT=attr_g[:, tt * P:(tt + 1) * P],
                                 rhs=we_sb[:], start=True, stop=True)
                wrelu = wpool.tile([P, IN_C * HID_C], fp16, tag="wrelu")
                eng = RELU_PATTERN[t % len(RELU_PATTERN)]
                if eng == "A":
                    nc.scalar.activation(wrelu[:], wpre[:], Act.Relu)
                else:
                    nc.vector.tensor_scalar(wrelu[:], wpre[:], 0.0, None,
                                            op0=Alu.max)
                prod = wpool.tile([P, IN_C * HID_C], fp16, tag="prod")
                xj_b = xj_g[:, t % G, :].unsqueeze(1).to_broadcast(
                    [P, HID_C, IN_C])
                prod_3d = prod[:].rearrange("p (h i) -> p h i", h=HID_C)
                wrelu_3d = wrelu[:].rearrange("p (h i) -> p h i", h=HID_C)
                if MUL_PATTERN[t % len(MUL_PATTERN)] == "G":
                    nc.gpsimd.tensor_tensor(prod_3d, wrelu_3d, xj_b,
                                            op=Alu.mult)
                else:
                    nc.vector.tensor_tensor(prod_3d, wrelu_3d, xj_b,
                                            op=Alu.mult)
                edge_state[t] = (prod, g, tt)

            def emit_scatter(t):
                k = int(tile_slot[t])
                kt = int(tile_kt[t])
                sb, kb = divmod(k, SBW)
                prod, g, tt = edge_state.pop(t)
                oh_g = group_state[g][2]
                if kb == 0 and kt == 0:
                    root_by_sb[sb] = root_ps.tile([P, SBW * HID_C], f32,
                                                  tag="root", name="root")
                if kt == 0:
                    agg_by_blk[k] = agg_ps.tile([P, IN_C * HID_C], f32,
                                                tag="agg", name="agg")
                    nsl = slice(k * P, (k + 1) * P)
                    nc.tensor.matmul(
                        root_by_sb[sb][:, kb * HID_C:(kb + 1) * HID_C],
                        lhsT=xsT_sb[:, nsl], rhs=wroot_sb[:],
                        start=True, stop=True)
                nc.tensor.matmul(agg_by_blk[k][:], lhsT=oh_g[:, tt, :],
                                 rhs=prod[:],
                                 start=(kt == 0), stop=(kt == kts[k] - 1))

            def emit_block_end(k):
                """Right after block k's last scatter: expanded i-reduce and
                fused h-add + LN stats, freeing the agg psum quickly."""
                sb, b = divmod(k, SBW)
                if b == 0:
                    h_sb = npool.tile([P, SBW * HID_C], f32, tag="h_sb",
                                      name="h_sb")
                    red = npool.tile([P, SBW * HID_C], f32, tag="red",
                                     name="red")
                    musum = spool.tile([P, SBW], f32, tag="musum",
                                       name="musum")
                    m2 = spool.tile([P, SBW], f32, tag="m2", name="m2")
                    hsq = wpool.tile([P, SBW * HID_C], fp16, tag="hsq",
                                     name="hsq")
                    sbt[sb] = {"h_sb": h_sb, "red": red, "musum": musum,
                               "m2": m2, "hsq": hsq}
                st = sbt[sb]
                root = root_by_sb[sb]
                agg = agg_by_blk.pop(k)
                hsl = slice(b * HID_C, (b + 1) * HID_C)
                nc.vector.tensor_reduce(
                    st["red"][:, hsl],
                    agg[:].rearrange("p (h i) -> p h i", h=HID_C),
                    axis=mybir.AxisListType.X, op=Alu.add)
                nc.vector.tensor_add(st["h_sb"][:, hsl], st["red"][:, hsl],
                                     root[:, hsl])
                nc.scalar.activation(st["hsq"][:, hsl], st["h_sb"][:, hsl],
                                     Act.Square,
                                     accum_out=st["m2"][:, b:b + 1])

            def emit_node_a(sb):
                """Batched LN scalar chain + fused relu((h-mu)*rstd)."""
                nblk = min(SBW, NB - sb * SBW)
                root_by_sb.pop(sb)
                stt = sbt.pop(sb)
                st = {}
                h_sb = stt["h_sb"]
                musum = stt["musum"]
                m2 = stt["m2"]
                nc.vector.tensor_reduce(
                    musum[:, :nblk],
                    h_sb[:, :nblk * HID_C].rearrange("p (b h) -> p b h",
                                                     h=HID_C),
                    axis=mybir.AxisListType.X, op=Alu.add)
                mu = spool.tile([P, SBW], f32, tag="mu")
                nc.vector.tensor_scalar(mu[:, :nblk], musum[:, :nblk],
                                        1.0 / HID_C, None, op0=Alu.mult)
                musq = spool.tile([P, SBW], f32, tag="musq")
                nc.vector.tensor_tensor(musq[:, :nblk], mu[:, :nblk],
                                        mu[:, :nblk], op=Alu.mult)
                m2n = spool.tile([P, SBW], f32, tag="m2n")
                nc.vector.tensor_scalar(m2n[:, :nblk], m2[:, :nblk],
                                        1.0 / HID_C, None, op0=Alu.mult)
                var = spool.tile([P, SBW], f32, tag="var")
                nc.vector.tensor_tensor(var[:, :nblk], m2n[:, :nblk],
                                        musq[:, :nblk], op=Alu.subtract)
                std = spool.tile([P, SBW], f32, tag="std")
                nc.scalar.activation(std[:, :nblk], var[:, :nblk], Act.Sqrt,
                                     bias=eps_c[:, :1])
                rstd = spool.tile([P, SBW], f32, tag="rstd")
                nc.vector.reciprocal(rstd[:, :nblk], std[:, :nblk])
                nmr = spool.tile([P, SBW], f32, tag="nmr")
                nc.vector.tensor_tensor(nmr[:, :nblk], mu[:, :nblk],
                                        rstd[:, :nblk], op=Alu.mult)
                nmrn = spool.tile([P, SBW], f32, tag="nmrn")
                nc.vector.tensor_scalar(nmrn[:, :nblk], nmr[:, :nblk],
                                        -1.0, None, op0=Alu.mult)
                hrelu = npool.tile([P, SBW * HID_C], fp16, tag="hrelu")
                for b in range(nblk):
                    hsl = slice(b * HID_C, (b + 1) * HID_C)
                    if not affine_ln:
                        nc.scalar.activation(hrelu[:, hsl], h_sb[:, hsl],
                                             Act.Relu,
                                             bias=nmrn[:, b:b + 1],
                                             scale=rstd[:, b:b + 1])
                    else:
                        nc.scalar.activation(hrelu[:, hsl], h_sb[:, hsl],
                                             Act.Copy,
                                             bias=0.0,
                                             scale=rstd[:, b:b + 1])
                if affine_ln:
                    # hrelu currently holds h*rstd; finish (x-mu)*rstd*g + b
                    # as ((h*rstd) + (-mu*rstd)) * gamma + beta, then relu.
                    sl = slice(0, nblk * HID_C)
                    t1 = npool.tile([P, SBW * HID_C], fp16, tag="at1")
                    nmr3 = nmrn[:, :nblk].unsqueeze(2).to_broadcast(
                        [P, nblk, HID_C])
                    nc.vector.tensor_tensor(
                        t1[:, sl].rearrange("p (b h) -> p b h", h=HID_C),
                        hrelu[:, sl].rearrange("p (b h) -> p b h", h=HID_C),
                        nmr3, op=Alu.add)
                    t2 = npool.tile([P, SBW * HID_C], fp16, tag="at2")
                    nc.vector.tensor_tensor(t2[:, sl], t1[:, sl],
                                            gam_sb[:, sl], op=Alu.mult)
                    t3 = npool.tile([P, SBW * HID_C], fp16, tag="at3")
                    nc.vector.tensor_tensor(t3[:, sl], t2[:, sl],
                                            bet_sb[:, sl], op=Alu.add)
                    nc.vector.tensor_scalar(hrelu[:, sl], t3[:, sl], 0.0,
                                            None, op0=Alu.max)
                st["hrelu"] = hrelu
                st["nblk"] = nblk
                sb_state[sb] = st

            def emit_node_b(sb):
                """Transpose + MLP head + output DMA for super-block sb."""
                st = sb_state.pop(sb)
                nblk = st["nblk"]
                hrelu = st.pop("hrelu")
                trp = tr_ps.tile([HID_C, SBW * P], fp16, tag="tr")
                for b in range(nblk):
                    nc.tensor.transpose(
                        trp[:, b * P:(b + 1) * P],
                        hrelu[:, b * HID_C:(b + 1) * HID_C], ident[:])
                hT = npool.tile([HID_C, SBW * P], fp16, tag="hT")
                nc.vector.tensor_copy(hT[:, :nblk * P], trp[:, :nblk * P])
                np_ = nblk * P
                fps = mlp_ps.tile([OUT_C, SBW * P], f32, tag="mlp",
                                  name="fps")
                nc.tensor.matmul(fps[:, :np_], lhsT=wlin_sb[:],
                                 rhs=hT[:, :np_], start=True, stop=True)
                fT = npool.tile([OUT_C, SBW * P], fp16, tag="fTs")
                nc.scalar.copy(fT[:, :np_], fps[:, :np_])
                q1ps = mlp_ps.tile([MLP_H, SBW * P], f32, tag="mlp",
                                   name="q1ps")
                nc.tensor.matmul(q1ps[:, :np_], lhsT=wq1_sb[:],
                                 rhs=fT[:, :np_], start=True, stop=True)
                q1r = npool.tile([MLP_H, SBW * P], fp16, tag="q1r")
                nc.vector.tensor_scalar(q1r[:, :np_], q1ps[:, :np_],
                                        bq1_sb[:, :1], 0.0,
                                        op0=Alu.add, op1=Alu.max)
                q2ps = mlp_ps.tile([N_ACT, SBW * P], f32, tag="mlp",
                                   name="q2ps")
                nc.tensor.matmul(q2ps[:, :np_], lhsT=wq2_sb[:],
                                 rhs=q1r[:, :np_], start=True, stop=True)
                qfin = npool.tile([N_ACT, SBW * P], f32, tag="qfin")
                nc.vector.tensor_scalar(qfin[:, :np_], q2ps[:, :np_],
                                        bq2_sb[:, :1], None, op0=Alu.add)
                csl = slice(sb * SBW * P, (sb * SBW + nblk) * P)
                nc.sync.dma_start(qT.ap()[:, csl], qfin[:, :np_])

            # ---- main pipeline ----
            sb_done = -1          # last super-block whose A stage was emitted
            for s in range(ET + EDGE_LAG):
                if s < ET:
                    emit_front(s)
                t = s - EDGE_LAG
                if t < 0:
                    continue
                emit_scatter(t)
                k = int(tile_slot[t])
                kt = int(tile_kt[t])
                if kt == kts[k] - 1:
                    emit_block_end(k)
                    if k % SBW == SBW - 1 or k == NB - 1:
                        sb = k // SBW
                        emit_node_a(sb)
                        if sb >= 1:
                            emit_node_b(sb - 1)
                        sb_done = sb
            emit_node_b(sb_done)

    nc.compile()
    return nc, ET


def _get_program(kts: tuple, affine_ln: bool):
    key = (kts, affine_ln, RELU_PATTERN, MUL_PATTERN)
    if key not in _PROGRAM_CACHE:
        _PROGRAM_CACHE[key] = _build_program(kts, affine_ln)
    return _PROGRAM_CACHE[key]


def _prep_inputs(x, edge_src, edge_dst, edge_attr,
                 We, be, Wroot, bconv, gamma, beta,
                 Wlin, blin, Wq1, bq1, Wq2, bq2):
    """Host-side sharding: bucket+sort edges by destination block, order block
    slots by count, pad to shared per-slot capacities, build per-core input
    maps.  Index/layout work only."""
    f32 = np.float32
    x = np.asarray(x, f32)
    edge_src = np.asarray(edge_src)
    edge_dst = np.asarray(edge_dst)
    edge_attr = np.asarray(edge_attr, f32)

    order = np.argsort(edge_dst, kind="stable")
    dst_s = edge_dst[order]
    src_s = edge_src[order]
    attr_s = edge_attr[order]

    core_of = dst_s // NPC
    local = dst_s - core_of * NPC
    blk = local // P
    gblk = core_of * NB + blk
    counts = np.bincount(gblk, minlength=M * NB).reshape(M, NB)

    # order slots by per-core descending count; shared per-slot capacities
    perm = np.argsort(-counts, axis=1, kind="stable")      # [M, NB]
    sorted_counts = np.take_along_axis(counts, perm, axis=1)
    kts = np.maximum(1, -(-sorted_counts.max(axis=0) // P))  # [NB] tiles
    kts_t = tuple(int(v) for v in kts)
    offs = np.concatenate([[0], np.cumsum(kts)]).astype(np.int64) * P
    EPC = int(offs[-1])

    # slot index of each block per core
    slot_of_blk = np.empty((M, NB), np.int64)
    np.put_along_axis(slot_of_blk, perm, np.arange(NB)[None, :], axis=1)

    # position of each edge in its core's padded edge array
    slot = slot_of_blk[core_of, blk]                        # [E]
    starts = np.zeros(M * NB, np.int64)
    starts[1:] = np.cumsum(counts.reshape(-1))[:-1]
    rank = np.arange(E, dtype=np.int64) - starts[gblk]
    pos = offs[slot] + rank                                 # within core
    gpos = core_of.astype(np.int64) * EPC + pos

    tot = M * EPC
    attr_all = np.zeros((tot, AK), np.float16)
    attr_all[gpos, :EDGE_D] = attr_s
    attr_all[gpos, EDGE_D] = 1.0
    xj_all = np.zeros((tot, IN_C), np.float16)
    xj_all[gpos] = x[src_s].astype(np.float16)
    oh_all = np.zeros((tot, P), np.float16)
    oh_all[gpos, local - blk * P] = 1.0

    attr_all = attr_all.reshape(M, EPC, AK)
    xj_all = xj_all.reshape(M, EPC, IN_C)
    oh_all = oh_all.reshape(M, EPC, P)

    # node features per slot order, augmented with ones row
    x_pad = np.zeros((M, NPC_PAD, IN_C + 1), np.float16)
    for c in range(M):
        for k in range(NB):
            b = perm[c, k]
            lo = c * NPC + b * P
            nb_sz = min(P, NPC - b * P)
            x_pad[c, k * P:k * P + nb_sz, :IN_C] = x[lo:lo + nb_sz]
    x_pad[:, :, IN_C] = 1.0

    # parameters (replicated)
    We = np.asarray(We, f32)
    be = np.asarray(be, f32)
    Wroot = np.asarray(Wroot, f32)
    bconv = np.asarray(bconv, f32)
    gamma = np.asarray(gamma, f32)
    beta = np.asarray(beta, f32)
    Wlin = np.asarray(Wlin, f32)
    blin = np.asarray(blin, f32)
    Wq1 = np.asarray(Wq1, f32)
    bq1 = np.asarray(bq1, f32)
    Wq2 = np.asarray(Wq2, f32)
    bq2 = np.asarray(bq2, f32)

    affine_ln = not (np.all(gamma == 1.0) and np.all(beta == 0.0))

    weA = np.concatenate([We, be[None, :]], axis=0)            # [9, 512]
    # permute columns from (i, h) to (h, i) layout so the i-contraction in
    # the node phase reduces over the contiguous innermost dim
    weA_perm = np.zeros((AK, IN_C * HID_C), np.float16)
    weA_perm[:EDGE_D + 1] = np.ascontiguousarray(
        weA.reshape(EDGE_D + 1, IN_C, HID_C).transpose(0, 2, 1)
           .reshape(EDGE_D + 1, IN_C * HID_C)).astype(np.float16)
    wrootA = np.concatenate([Wroot, bconv[None, :]], axis=0).astype(np.float16)
    bq1p = (blin @ Wq1 + bq1).astype(f32)                      # blin folded
    gam4 = np.broadcast_to(np.tile(gamma, SBW), (P, SBW * HID_C)).astype(
        np.float16).copy()
    bet4 = np.broadcast_to(np.tile(beta, SBW), (P, SBW * HID_C)).astype(
        np.float16).copy()

    in_maps = []
    for c in range(M):
        in_maps.append({
            "attrT": np.ascontiguousarray(attr_all[c].T),
            "xjg": np.ascontiguousarray(xj_all[c]),
            "ohg": np.ascontiguousarray(oh_all[c]),
            "xsT": np.ascontiguousarray(x_pad[c].T),
            "weA": weA_perm,
            "wrootA": wrootA,
            "wlin": Wlin.astype(np.float16),
            "wq1": Wq1.astype(np.float16),
            "wq2": Wq2.astype(np.float16),
            "bq1c": bq1p[:, None],
            "bq2c": bq2[:, None],
            "gamma4": gam4,
            "beta4": bet4,
        })
    return kts_t, affine_ln, perm, in_maps


def kernel(**inputs) -> np.ndarray:
    from concourse.bass_utils import run_bass_kernel_spmd

    kts_t, affine_ln, perm, in_maps = _prep_inputs(**inputs)
    nc, _ = _get_program(kts_t, affine_ln)
    res = run_bass_kernel_spmd(nc, in_maps, list(range(M)))
    q = np.empty((N, N_ACT), np.float32)
    for c in range(M):
        qTc = res.results[c]["qT"]
        for k in range(NB):
            b = int(perm[c, k])
            nb_sz = min(P, NPC - b * P)
            q[c * NPC + b * P: c * NPC + b * P + nb_sz] = \
                qTc[:, k * P:k * P + nb_sz].T
    return q


# revision 14
# speedup vs baseline: 1.7691x; 1.0843x over previous
"""Trainium2 Bass kernel for the DiscreteAgent GNN (NNConv + LN + MLP head).

Strategy (8 NeuronCores, SPMD, no collectives):
  * Edges bucketed by destination 128-node block; each core owns a disjoint
    6250-node range so outputs never overlap -> no all-reduce.
  * Within a core, block slots are ordered by descending edge count and the
    per-slot tile capacity is the max across cores, so the SPMD program is
    shared while padding stays small.
  * Per 128-edge tile: PE computes w_pre = [attr|1]^T @ [We;be] (512 cols),
    ACT/GPSIMD/DVE apply relu (split by tile for balance), DVE multiplies by
    the broadcast source features, and the i-contraction is FUSED into the
    scatter: 16 chained 32-col matmuls (one per input channel) accumulate
    onehot^T @ prod[:, :, i] into the block's [128, 32] PSUM slice.  The
    root-weight matmul opens each block's accumulation chain, so
    h = agg + x@Wroot + bconv materializes directly in PSUM.
  * One-hot scatter matrices are precomputed on the host and DMA'd (no
    per-tile is_equal on DVE).
  * Node phase per 4-block super-block: batched LN stats, per-block
    relu((h-mu)*rstd) fused into one ACT op (scale/bias per-partition),
    batched PE transpose, then the 3-layer MLP head in 2-block chunks with
    dual-op PSUM drains.
"""

import sys

import numpy as np

try:
    import concourse  # noqa: F401
except ImportError:  # pragma: no cover
    for _p in ("/opt/trn_rl_repo", "/opt/pypackages"):
        if _p not in sys.path:
            sys.path.insert(0, _p)

# ---- problem constants (hardcoded per contract) ----
N = 50000
E = 200000
IN_C = 16
HID_C = 32
EDGE_D = 8
OUT_C = 32
MLP_H = 128
N_ACT = 32

M = 8                 # cores
P = 128               # partitions
AK = 128              # contraction rows for the w_pre matmul (zero-padded:
                      # K=9 matmuls stream at half rate on HW)
NPC = N // M          # 6250 nodes per core
NB = (NPC + P - 1) // P   # 49 block slots per core
NPC_PAD = NB * P      # 6272
G = 8                 # edge tiles per DMA group
SBW = 4               # blocks per node-phase super-block (stats/transpose)
CW = 2                # blocks per MLP chunk
EDGE_LAG = 6          # scatter trails the relu/mul front by this many tiles

# relu engine per tile, cycled: 'A'=ACT, 'V'=DVE (GPSIMD cannot read PSUM)
RELU_PATTERN = "AAAAAVAAAAAV"
# mul engine per tile: 'V'=DVE, 'G'=GPSIMD
MUL_PATTERN = "GVGVGVGVGVGV"

_PROGRAM_CACHE: dict = {}


def _build_program(kts: tuple, affine_ln: bool):
    """Build + compile the SPMD Bass program.

    kts: per-block-slot edge-tile counts (len NB, each >= 1).
    affine_ln: if True, gamma/beta are non-trivial and applied explicitly.
    """
    import concourse.tile as tile
    from concourse import bacc, mybir
    from concourse.masks import make_identity

    f32 = mybir.dt.float32
    fp16 = mybir.dt.float16
    Act = mybir.ActivationFunctionType
    Alu = mybir.AluOpType

    ET = int(sum(kts))            # edge tiles per core
    EPC = ET * P                  # padded edge slots per core
    offs = np.concatenate([[0], np.cumsum(kts)]).astype(int)  # tile offsets

    # flat tile -> (slot, kt) map
    tile_slot = np.empty(ET, int)
    tile_kt = np.empty(ET, int)
    for k in range(NB):
        tile_slot[offs[k]:offs[k + 1]] = k
        tile_kt[offs[k]:offs[k + 1]] = np.arange(kts[k])

    nc = bacc.Bacc("TRN2", target_bir_lowering=False, debug=False, num_devices=M)

    # --- DRAM I/O (per core) ---
    attrT = nc.dram_tensor("attrT", [AK, EPC], fp16, kind="ExternalInput")
    xjg = nc.dram_tensor("xjg", [EPC, IN_C], fp16, kind="ExternalInput")
    ohg = nc.dram_tensor("ohg", [EPC, P], fp16, kind="ExternalInput")
    xsT = nc.dram_tensor("xsT", [IN_C + 1, NPC_PAD], fp16, kind="ExternalInput")
    weA = nc.dram_tensor("weA", [AK, IN_C * HID_C], fp16, kind="ExternalInput")
    wrootA = nc.dram_tensor("wrootA", [IN_C + 1, HID_C], fp16, kind="ExternalInput")
    wlin = nc.dram_tensor("wlin", [HID_C, OUT_C], fp16, kind="ExternalInput")
    wq1 = nc.dram_tensor("wq1", [OUT_C, MLP_H], fp16, kind="ExternalInput")
    wq2 = nc.dram_tensor("wq2", [MLP_H, N_ACT], fp16, kind="ExternalInput")
    bq1c = nc.dram_tensor("bq1c", [MLP_H, 1], f32, kind="ExternalInput")
    bq2c = nc.dram_tensor("bq2c", [N_ACT, 1], f32, kind="ExternalInput")
    gamma4 = nc.dram_tensor("gamma4", [P, SBW * HID_C], fp16, kind="ExternalInput")
    beta4 = nc.dram_tensor("beta4", [P, SBW * HID_C], fp16, kind="ExternalInput")
    qT = nc.dram_tensor("qT", [N_ACT, NPC_PAD], f32, kind="ExternalOutput")

    NSB = (NB + SBW - 1) // SBW   # super-blocks

    with tile.TileContext(nc) as tc:
        with (
            tc.tile_pool(name="const", bufs=1) as cpool,
            tc.tile_pool(name="edge_in", bufs=3) as epool,
            tc.tile_pool(name="wrelu", bufs=9) as wpool,
            tc.tile_pool(name="node", bufs=2) as npool,
            tc.tile_pool(name="stats", bufs=2) as spool,
            tc.tile_pool(name="wpre_ps", bufs=3, space="PSUM") as wpre_ps,
            tc.tile_pool(name="agg_ps", bufs=2, space="PSUM") as agg_ps,
            tc.tile_pool(name="root_ps", bufs=1, space="PSUM") as root_ps,
            tc.tile_pool(name="mlp_ps", bufs=2, space="PSUM") as mlp_ps,
        ):
            group_state = {}
            NGROUPS = (ET + G - 1) // G

            def emit_group_load(g):
                gs = min(G, ET - g * G)
                esl = slice(g * G * P, (g * G + gs) * P)
                attr_g = epool.tile([AK, G * P], fp16, tag="attr")
                nc.sync.dma_start(attr_g[:, :gs * P], attrT.ap()[:, esl])
                xj_g = epool.tile([P, G, IN_C], fp16, tag="xj")
                nc.sync.dma_start(
                    xj_g[:, :gs, :],
                    xjg.ap()[esl, :].rearrange("(tt p) i -> p tt i", p=P))
                oh_g = epool.tile([P, G, P], fp16, tag="oh")
                nc.sync.dma_start(
                    oh_g[:, :gs, :],
                    ohg.ap()[esl, :].rearrange("(tt p) n -> p tt n", p=P))
                group_state[g] = (attr_g, xj_g, oh_g)

            for g in range(2):
                emit_group_load(g)

            # ---- persistent constants in SBUF ----
            we_sb = cpool.tile([AK, IN_C * HID_C], fp16, tag="we")
            nc.sync.dma_start(we_sb[:], weA.ap()[:])
            xsT_sb = cpool.tile([IN_C + 1, NPC_PAD], fp16, tag="xsT")
            nc.sync.dma_start(xsT_sb[:], xsT.ap()[:])
            wroot_sb = cpool.tile([IN_C + 1, HID_C], fp16, tag="wroot")
            nc.sync.dma_start(wroot_sb[:], wrootA.ap()[:])
            wlin_sb = cpool.tile([HID_C, OUT_C], fp16, tag="wlin")
            nc.sync.dma_start(wlin_sb[:], wlin.ap()[:])
            wq1_sb = cpool.tile([OUT_C, MLP_H], fp16, tag="wq1")
            nc.sync.dma_start(wq1_sb[:], wq1.ap()[:])
            wq2_sb = cpool.tile([MLP_H, N_ACT], fp16, tag="wq2")
            nc.sync.dma_start(wq2_sb[:], wq2.ap()[:])
            bq1_sb = cpool.tile([MLP_H, 1], f32, tag="bq1")
            nc.sync.dma_start(bq1_sb[:], bq1c.ap()[:])
            bq2_sb = cpool.tile([N_ACT, 1], f32, tag="bq2")
            nc.sync.dma_start(bq2_sb[:], bq2c.ap()[:])
            if affine_ln:
                gam_sb = cpool.tile([P, SBW * HID_C], fp16, tag="gam")
                nc.sync.dma_start(gam_sb[:], gamma4.ap()[:])
                bet_sb = cpool.tile([P, SBW * HID_C], fp16, tag="bet")
                nc.sync.dma_start(bet_sb[:], beta4.ap()[:])

            ident = cpool.tile([P, P], fp16, tag="ident")
            make_identity(nc, ident[:])
            eps_c = cpool.tile([P, 1], f32, tag="eps")
            nc.gpsimd.memset(eps_c[:], 1e-5)

            # ---- pipeline state ----
            edge_state = {}       # t -> prod tile
            agg_by_blk = {}       # k -> expanded agg psum tile [P, 512]
            root_by_sb = {}       # sb -> root psum tile [P, SBW*HID_C]
            sbt = {}              # sb -> dict of stat tiles (filled per block)
            sb_state = {}         # sb -> dict (stage A outputs)

            def emit_front(t):
                g, tt = divmod(t, G)
                if tt == 0 and g not in group_state:
                    emit_group_load(g)
                if tt == 0 and g + 1 < NGROUPS and g + 1 not in group_state:
                    emit_group_load(g + 1)
                attr_g, xj_g, oh_g = group_state[g]

                wpre = wpre_ps.tile([P, IN_C * HID_C], f32, tag="wpre")
                nc.tensor.matmul(wpre[:], lhsT=attr_g[:, tt * P:(tt + 1) * P],
                                 rhs=we_sb[:], start=True, stop=True)
                wrelu = wpool.tile([P, IN_C * HID_C], fp16, tag="wrelu")
                eng = RELU_PATTERN[t % len(RELU_PATTERN)]
                if eng == "A":
                    nc.scalar.activation(wrelu[:], wpre[:], Act.Relu)
                else:
                    nc.vector.tensor_scalar(wrelu[:], wpre[:], 0.0, None,
                                            op0=Alu.max)
                prod = wpool.tile([P, IN_C * HID_C], fp16, tag="prod")
                xj_b = xj_g[:, t % G, :].unsqueeze(1).to_broadcast(
                    [P, HID_C, IN_C])
                prod_3d = prod[:].rearrange("p (h i) -> p h i", h=HID_C)
                wrelu_3d = wrelu[:].rearrange("p (h i) -> p h i", h=HID_C)
                if MUL_PATTERN[t % len(MUL_PATTERN)] == "G":
                    nc.gpsimd.tensor_tensor(prod_3d, wrelu_3d, xj_b,
                                            op=Alu.mult)
                else:
                    nc.vector.tensor_tensor(prod_3d, wrelu_3d, xj_b,
                                            op=Alu.mult)
                edge_state[t] = (prod, g, tt)

            def emit_scatter(t):
                k = int(tile_slot[t])
                kt = int(tile_kt[t])
                sb, kb = divmod(k, SBW)
                prod, g, tt = edge_state.pop(t)
                oh_g = group_state[g][2]
                if kb == 0 and kt == 0:
                    root_by_sb[sb] = root_ps.tile([P, SBW * HID_C], f32,
                                                  tag="root", name="root")
                if kt == 0:
                    agg_by_blk[k] = agg_ps.tile([P, IN_C * HID_C], f32,
                                                tag="agg", name="agg")
                    nsl = slice(k * P, (k + 1) * P)
                    nc.tensor.matmul(
                        root_by_sb[sb][:, kb * HID_C:(kb + 1) * HID_C],
                        lhsT=xsT_sb[:, nsl], rhs=wroot_sb[:],
                        start=True, stop=True)
                nc.tensor.matmul(agg_by_blk[k][:], lhsT=oh_g[:, tt, :],
                                 rhs=prod[:],
                                 start=(kt == 0), stop=(kt == kts[k] - 1))

            def emit_block_end(k):
                """Right after block k's last scatter: expanded i-reduce and
                fused h-add + LN stats, freeing the agg psum quickly."""
                sb, b = divmod(k, SBW)
                if b == 0:
                    h_sb = npool.tile([P, SBW * HID_C], f32, tag="h_sb",
                                      name="h_sb")
                    red = npool.tile([P, SBW * HID_C], f32, tag="red",
                                     name="red")
                    musum = spool.tile([P, SBW], f32, tag="musum",
                                       name="musum")
                    m2 = spool.tile([P, SBW], f32, tag="m2", name="m2")
                    hsq = wpool.tile([P, SBW * HID_C], f32, tag="hsq",
                                     name="hsq")
                    sbt[sb] = {"h_sb": h_sb, "red": red, "musum": musum,
                               "m2": m2, "hsq": hsq}
                st = sbt[sb]
                root = root_by_sb[sb]
                agg = agg_by_blk.pop(k)
                hsl = slice(b * HID_C, (b + 1) * HID_C)
                nc.vector.tensor_reduce(
                    st["red"][:, hsl],
                    agg[:].rearrange("p (h i) -> p h i", h=HID_C),
                    axis=mybir.AxisListType.X, op=Alu.add)
                nc.vector.tensor_add(st["h_sb"][:, hsl], st["red"][:, hsl],
                                     root[:, hsl])
                nc.gpsimd.tensor_tensor(st["hsq"][:, hsl], st["h_sb"][:, hsl],
                                        st["h_sb"][:, hsl], op=Alu.mult)

            def emit_node_a(sb):
                """Batched LN scalar chain + fused relu((h-mu)*rstd)."""
                nblk = min(SBW, NB - sb * SBW)
                root_by_sb.pop(sb)
                stt = sbt.pop(sb)
                st = {}
                h_sb = stt["h_sb"]
                musum = stt["musum"]
                m2 = stt["m2"]
                nc.vector.tensor_reduce(
                    musum[:, :nblk],
                    h_sb[:, :nblk * HID_C].rearrange("p (b h) -> p b h",
                                                     h=HID_C),
                    axis=mybir.AxisListType.X, op=Alu.add)
                nc.vector.tensor_reduce(
                    m2[:, :nblk],
                    stt["hsq"][:, :nblk * HID_C].rearrange(
                        "p (b h) -> p b h", h=HID_C),
                    axis=mybir.AxisListType.X, op=Alu.add)
                mu = spool.tile([P, SBW], f32, tag="mu")
                nc.vector.tensor_scalar(mu[:, :nblk], musum[:, :nblk],
                                        1.0 / HID_C, None, op0=Alu.mult)
                musq = spool.tile([P, SBW], f32, tag="musq")
                nc.vector.tensor_tensor(musq[:, :nblk], mu[:, :nblk],
                                        mu[:, :nblk], op=Alu.mult)
                m2n = spool.tile([P, SBW], f32, tag="m2n")
                nc.vector.tensor_scalar(m2n[:, :nblk], m2[:, :nblk],
                                        1.0 / HID_C, None, op0=Alu.mult)
                var = spool.tile([P, SBW], f32, tag="var")
                nc.vector.tensor_tensor(var[:, :nblk], m2n[:, :nblk],
                                        musq[:, :nblk], op=Alu.subtract)
                std = spool.tile([P, SBW], f32, tag="std")
                nc.scalar.activation(std[:, :nblk], var[:, :nblk], Act.Sqrt,
                                     bias=eps_c[:, :1])
                rstd = spool.tile([P, SBW], f32, tag="rstd")
                nc.vector.reciprocal(rstd[:, :nblk], std[:, :nblk])
                nmr = spool.tile([P, SBW], f32, tag="nmr")
                nc.vector.tensor_tensor(nmr[:, :nblk], mu[:, :nblk],
                                        rstd[:, :nblk], op=Alu.mult)
                nmrn = spool.tile([P, SBW], f32, tag="nmrn")
                nc.vector.tensor_scalar(nmrn[:, :nblk], nmr[:, :nblk],
                                        -1.0, None, op0=Alu.mult)
                hrelu = npool.tile([P, SBW * HID_C], fp16, tag="hrelu")
                for b in range(nblk):
                    hsl = slice(b * HID_C, (b + 1) * HID_C)
                    if not affine_ln:
                        nc.scalar.activation(hrelu[:, hsl], h_sb[:, hsl],
                                             Act.Relu,
                                             bias=nmrn[:, b:b + 1],
                                             scale=rstd[:, b:b + 1])
                    else:
                        nc.scalar.activation(hrelu[:, hsl], h_sb[:, hsl],
                                             Act.Copy,
                                             bias=0.0,
                                             scale=rstd[:, b:b + 1])
                if affine_ln:
                    # hrelu currently holds h*rstd; finish (x-mu)*rstd*g + b
                    # as ((h*rstd) + (-mu*rstd)) * gamma + beta, then relu.
                    sl = slice(0, nblk * HID_C)
                    t1 = npool.tile([P, SBW * HID_C], fp16, tag="at1")
                    nmr3 = nmrn[:, :nblk].unsqueeze(2).to_broadcast(
                        [P, nblk, HID_C])
                    nc.vector.tensor_tensor(
                        t1[:, sl].rearrange("p (b h) -> p b h", h=HID_C),
                        hrelu[:, sl].rearrange("p (b h) -> p b h", h=HID_C),
                        nmr3, op=Alu.add)
                    t2 = npool.tile([P, SBW * HID_C], fp16, tag="at2")
                    nc.vector.tensor_tensor(t2[:, sl], t1[:, sl],
                                            gam_sb[:, sl], op=Alu.mult)
                    t3 = npool.tile([P, SBW * HID_C], fp16, tag="at3")
                    nc.vector.tensor_tensor(t3[:, sl], t2[:, sl],
                                            bet_sb[:, sl], op=Alu.add)
                    nc.vector.tensor_scalar(hrelu[:, sl], t3[:, sl], 0.0,
                                            None, op0=Alu.max)
                st["hrelu"] = hrelu
                st["nblk"] = nblk
                sb_state[sb] = st

            def emit_node_b(sb):
                """Transpose + MLP head + output DMA for super-block sb."""
                st = sb_state.pop(sb)
                nblk = st["nblk"]
                hrelu = st.pop("hrelu")
                trp = mlp_ps.tile([HID_C, SBW * P], fp16, tag="mlp",
                                  name="trp")
                for b in range(nblk):
                    nc.tensor.transpose(
                        trp[:, b * P:(b + 1) * P],
                        hrelu[:, b * HID_C:(b + 1) * HID_C], ident[:])
                hT = npool.tile([HID_C, SBW * P], fp16, tag="hT")
                nc.vector.tensor_copy(hT[:, :nblk * P], trp[:, :nblk * P])
                np_ = nblk * P
                fps = mlp_ps.tile([OUT_C, SBW * P], f32, tag="mlp",
                                  name="fps")
                nc.tensor.matmul(fps[:, :np_], lhsT=wlin_sb[:],
                                 rhs=hT[:, :np_], start=True, stop=True)
                fT = npool.tile([OUT_C, SBW * P], fp16, tag="fTs")
                nc.vector.tensor_copy(fT[:, :np_], fps[:, :np_])
                q1ps = mlp_ps.tile([MLP_H, SBW * P], f32, tag="mlp",
                                   name="q1ps")
                nc.tensor.matmul(q1ps[:, :np_], lhsT=wq1_sb[:],
                                 rhs=fT[:, :np_], start=True, stop=True)
                q1r = npool.tile([MLP_H, SBW * P], fp16, tag="q1r")
                nc.vector.tensor_scalar(q1r[:, :np_], q1ps[:, :np_],
                                        bq1_sb[:, :1], 0.0,
                                        op0=Alu.add, op1=Alu.max)
                q2ps = mlp_ps.tile([N_ACT, SBW * P], f32, tag="mlp",
                                   name="q2ps")
                nc.tensor.matmul(q2ps[:, :np_], lhsT=wq2_sb[:],
                                 rhs=q1r[:, :np_], start=True, stop=True)
                qfin = npool.tile([N_ACT, SBW * P], f32, tag="qfin")
                nc.vector.tensor_scalar(qfin[:, :np_], q2ps[:, :np_],
                                        bq2_sb[:, :1], None, op0=Alu.add)
                csl = slice(sb * SBW * P, (sb * SBW + nblk) * P)
                nc.sync.dma_start(qT.ap()[:, csl], qfin[:, :np_])

            # ---- main pipeline ----
            sb_done = -1          # last super-block whose A stage was emitted
            for s in range(ET + EDGE_LAG):
                if s < ET:
                    emit_front(s)
                t = s - EDGE_LAG
                if t < 0:
                    continue
                emit_scatter(t)
                k = int(tile_slot[t])
                kt = int(tile_kt[t])
                if kt == kts[k] - 1:
                    emit_block_end(k)
                    if k % SBW == SBW - 1 or k == NB - 1:
                        sb = k // SBW
                        emit_node_a(sb)
                        if sb >= 1:
                            emit_node_b(sb - 1)
                        sb_done = sb
            emit_node_b(sb_done)

    nc.compile()
    return nc, ET


def _get_program(kts: tuple, affine_ln: bool):
    key = (kts, affine_ln, RELU_PATTERN, MUL_PATTERN)
    if key not in _PROGRAM_CACHE:
        _PROGRAM_CACHE[key] = _build_program(kts, affine_ln)
    return _PROGRAM_CACHE[key]


def _prep_inputs(x, edge_src, edge_dst, edge_attr,
                 We, be, Wroot, bconv, gamma, beta,
                 Wlin, blin, Wq1, bq1, Wq2, bq2):
    """Host-side sharding: bucket+sort edges by destination block, order block
    slots by count, pad to shared per-slot capacities, build per-core input
    maps.  Index/layout work only."""
    f32 = np.float32
    x = np.asarray(x, f32)
    edge_src = np.asarray(edge_src)
    edge_dst = np.asarray(edge_dst)
    edge_attr = np.asarray(edge_attr, f32)

    order = np.argsort(edge_dst, kind="stable")
    dst_s = edge_dst[order]
    src_s = edge_src[order]
    attr_s = edge_attr[order]

    core_of = dst_s // NPC
    local = dst_s - core_of * NPC
    blk = local // P
    gblk = core_of * NB + blk
    counts = np.bincount(gblk, minlength=M * NB).reshape(M, NB)

    # order slots by per-core descending count; shared per-slot capacities
    perm = np.argsort(-counts, axis=1, kind="stable")      # [M, NB]
    sorted_counts = np.take_along_axis(counts, perm, axis=1)
    kts = np.maximum(1, -(-sorted_counts.max(axis=0) // P))  # [NB] tiles
    kts_t = tuple(int(v) for v in kts)
    offs = np.concatenate([[0], np.cumsum(kts)]).astype(np.int64) * P
    EPC = int(offs[-1])

    # slot index of each block per core
    slot_of_blk = np.empty((M, NB), np.int64)
    np.put_along_axis(slot_of_blk, perm, np.arange(NB)[None, :], axis=1)

    # position of each edge in its core's padded edge array
    slot = slot_of_blk[core_of, blk]                        # [E]
    starts = np.zeros(M * NB, np.int64)
    starts[1:] = np.cumsum(counts.reshape(-1))[:-1]
    rank = np.arange(E, dtype=np.int64) - starts[gblk]
    pos = offs[slot] + rank                                 # within core
    gpos = core_of.astype(np.int64) * EPC + pos

    tot = M * EPC
    attr_all = np.zeros((tot, AK), np.float16)
    attr_all[gpos, :EDGE_D] = attr_s
    attr_all[gpos, EDGE_D] = 1.0
    xj_all = np.zeros((tot, IN_C), np.float16)
    xj_all[gpos] = x[src_s].astype(np.float16)
    oh_all = np.zeros((tot, P), np.float16)
    oh_all[gpos, local - blk * P] = 1.0

    attr_all = attr_all.reshape(M, EPC, AK)
    xj_all = xj_all.reshape(M, EPC, IN_C)
    oh_all = oh_all.reshape(M, EPC, P)

    # node features per slot order, augmented with ones row
    x_pad = np.zeros((M, NPC_PAD, IN_C + 1), np.float16)
    for c in range(M):
        for k in range(NB):
            b = perm[c, k]
            lo = c * NPC + b * P
            nb_sz = min(P, NPC - b * P)
            x_pad[c, k * P:k * P + nb_sz, :IN_C] = x[lo:lo + nb_sz]
    x_pad[:, :, IN_C] = 1.0

    # parameters (replicated)
    We = np.asarray(We, f32)
    be = np.asarray(be, f32)
    Wroot = np.asarray(Wroot, f32)
    bconv = np.asarray(bconv, f32)
    gamma = np.asarray(gamma, f32)
    beta = np.asarray(beta, f32)
    Wlin = np.asarray(Wlin, f32)
    blin = np.asarray(blin, f32)
    Wq1 = np.asarray(Wq1, f32)
    bq1 = np.asarray(bq1, f32)
    Wq2 = np.asarray(Wq2, f32)
    bq2 = np.asarray(bq2, f32)

    affine_ln = not (np.all(gamma == 1.0) and np.all(beta == 0.0))

    weA = np.concatenate([We, be[None, :]], axis=0)            # [9, 512]
    # permute columns from (i, h) to (h, i) layout so the i-contraction in
    # the node phase reduces over the contiguous innermost dim
    weA_perm = np.zeros((AK, IN_C * HID_C), np.float16)
    weA_perm[:EDGE_D + 1] = np.ascontiguousarray(
        weA.reshape(EDGE_D + 1, IN_C, HID_C).transpose(0, 2, 1)
           .reshape(EDGE_D + 1, IN_C * HID_C)).astype(np.float16)
    wrootA = np.concatenate([Wroot, bconv[None, :]], axis=0).astype(np.float16)
    bq1p = (blin @ Wq1 + bq1).astype(f32)                      # blin folded
    gam4 = np.broadcast_to(np.tile(gamma, SBW), (P, SBW * HID_C)).astype(
        np.float16).copy()
    bet4 = np.broadcast_to(np.tile(beta, SBW), (P, SBW * HID_C)).astype(
        np.float16).copy()

    in_maps = []
    for c in range(M):
        in_maps.append({
            "attrT": np.ascontiguousarray(attr_all[c].T),
            "xjg": np.ascontiguousarray(xj_all[c]),
            "ohg": np.ascontiguousarray(oh_all[c]),
            "xsT": np.ascontiguousarray(x_pad[c].T),
            "weA": weA_perm,
            "wrootA": wrootA,
            "wlin": Wlin.astype(np.float16),
            "wq1": Wq1.astype(np.float16),
            "wq2": Wq2.astype(np.float16),
            "bq1c": bq1p[:, None],
            "bq2c": bq2[:, None],
            "gamma4": gam4,
            "beta4": bet4,
        })
    return kts_t, affine_ln, perm, in_maps


def kernel(**inputs) -> np.ndarray:
    from concourse.bass_utils import run_bass_kernel_spmd

    kts_t, affine_ln, perm, in_maps = _prep_inputs(**inputs)
    nc, _ = _get_program(kts_t, affine_ln)
    res = run_bass_kernel_spmd(nc, in_maps, list(range(M)))
    q = np.empty((N, N_ACT), np.float32)
    for c in range(M):
        qTc = res.results[c]["qT"]
        for k in range(NB):
            b = int(perm[c, k])
            nb_sz = min(P, NPC - b * P)
            q[c * NPC + b * P: c * NPC + b * P + nb_sz] = \
                qTc[:, k * P:k * P + nb_sz].T
    return q
